# revision 1
# baseline (speedup 1.0000x reference)
"""Trainium2 Bass kernel for nn_EquivariantLayer (spectral equivariant layer).

Strategy (data-parallel over batch, 2 samples/core x 8 cores):
  All FFTs are expressed as real DFT matmuls on the TensorEngine with layouts
  chosen so no corner-turn transposes are ever needed:

    stage1:  A = f^T @ [ExR^T | ExI^T]          (contract x; out [y, (RI,kx)])
    stage2:  F = Ey @ A                          (contract y; out [c, kx], c-major)
             -> two layouts: conv layout [(i%4)*32+c, kx] and fr layout [c, (i,kx)]
    conv:    M = F (*) K elementwise (K = rfft2(sym kernel) is REAL since the
             symmetrized kernel is D4-symmetric); i-reduction via a selector
             matmul on the TensorEngine (PSUM accumulation over i-halves)
    uncurl:  TO_U = i*t, TO_V = i*s are pure-imaginary -> 2 real mults each
    synth:   field = Re(P @ B^T_cm @ Q^T) as two matmul stages (stage a/b)
    cross:   u_a v_b - u_b v_a on the VectorEngine with zero-step broadcast APs

Output [16, 128, 128, 128] f32 (~134 MB) dominates traffic (memory regime).
"""
import sys
import numpy as np

if '/opt/trn_rl_repo' not in sys.path:
    sys.path.insert(0, '/opt/trn_rl_repo')

import concourse.bass as bass
from concourse import bacc
import concourse.mybir as mybir
import concourse.tile as tile
from concourse.bass import AP
from concourse.bass_utils import run_bass_kernel_spmd

F32 = mybir.dt.float32
N_CORES = 8
B_PER_CORE = 2
C1, C2, N1, N2 = 8, 16, 64, 128
NCH_OUT = 128  # 8 fr + 120 cross

I_IDX, J_IDX = np.triu_indices(C2, 1)
_PAIR_IDX = {}
for _p, (_a, _b) in enumerate(zip(I_IDX, J_IDX)):
    _PAIR_IDX[(int(_a), int(_b))] = _p


# ---------------------------------------------------------------------------
# host-side constant construction
# ---------------------------------------------------------------------------

def _host_consts():
    x = np.arange(64)
    kx = np.arange(64)
    c = np.arange(32)
    y = np.arange(64)
    X = np.arange(128)
    Y = np.arange(128)

    FRs = np.where(kx <= 32, kx, kx - 64).astype(np.float64)  # signed row freq

    ExR = np.cos(2 * np.pi * np.outer(kx, x) / 64)   # [kx, x]
    ExI = -np.sin(2 * np.pi * np.outer(kx, x) / 64)
    # [A_R | A_I | -A_R] so stage2 fuses R/I into two matmuls
    ExF = np.concatenate([ExR.T, ExI.T, -ExR.T], axis=1)   # [x, 192]

    # F_R = C A_R + S A_I ; F_I = C A_I + S (-A_R)   (C=cos, S=sin)
    EyCT = np.cos(2 * np.pi * np.outer(c, y) / 64).T   # [y=64, c=32]
    EyST = np.sin(2 * np.pi * np.outer(c, y) / 64).T

    S_sel = np.zeros((128, 32))
    for im in range(4):
        S_sel[im * 32 + np.arange(32), np.arange(32)] = 1.0

    den = FRs[None, :] ** 2 + c[:, None].astype(np.float64) ** 2
    den[0, 0] = 1.0
    t_u = c[:, None] / den                           # [32, 64]
    s_v = -FRs[None, :] / den
    t_rep = np.tile(t_u, (1, 8))                     # [32, 512] (j-rep)
    s_rep = np.tile(s_v, (1, 8))
    tsg = np.concatenate([-t_rep, t_rep, -s_rep, s_rep], axis=1)  # [32, 2048]

    w_c = np.where(c == 0, 1.0, 2.0)
    s_q = 2.0 / (128.0 * 128.0)
    QRT = (s_q * w_c[None, :] * np.cos(2 * np.pi * np.outer(Y, c) / 128)).T  # [c, Y]
    QIT = (s_q * w_c[None, :] * np.sin(2 * np.pi * np.outer(Y, c) / 128)).T
    QF1 = np.concatenate([QRT, QIT], axis=1)         # [32, 256]
    QF2 = np.concatenate([-QIT, QRT], axis=1)

    PRT = np.cos(2 * np.pi * np.outer(FRs, X) / 128)   # [r=64, X=128]
    PIT = np.sin(2 * np.pi * np.outer(FRs, X) / 128)
    PRT[32, :] = 0.0
    PIT[32, :] = 0.0
    PRTPnIT = np.concatenate([PRT, -PIT], axis=0)    # [128, 128] (K-stacked)

    # direct fr path: fr_i = Rx @ f_i @ Cy^T (pure 2x Fourier upsampling)
    ExRm = np.cos(2 * np.pi * np.outer(kx, x) / 64)
    ExIm = -np.sin(2 * np.pi * np.outer(kx, x) / 64)
    EyRm = np.cos(2 * np.pi * np.outer(c, y) / 64)
    EyIm = -np.sin(2 * np.pi * np.outer(c, y) / 64)
    QRm = s_q * w_c[None, :] * np.cos(2 * np.pi * np.outer(Y, c) / 128)
    QIm = s_q * w_c[None, :] * np.sin(2 * np.pi * np.outer(Y, c) / 128)
    Rx = PRT.T @ ExRm - PIT.T @ ExIm                 # [128, 64] (PRT.T == PR)
    Cy = QRm @ EyRm - QIm @ EyIm                     # [128, 64]
    RxT = Rx.T                                       # [x=64, X=128]
    CyT = np.concatenate([Cy.T, Cy.T], axis=0)       # [128, 128] doubled rows

    f32 = lambda a: np.ascontiguousarray(a, dtype=np.float32)
    return dict(ExF=f32(ExF), EyCT=f32(EyCT), EyST=f32(EyST),
                S_sel=f32(S_sel), tsg=f32(tsg), QF1=f32(QF1), QF2=f32(QF2),
                PRTPnIT=f32(PRTPnIT), RxT=f32(RxT), CyT=f32(CyT))


def _rot90_kernel(k):
    # z[..., i, j] = k[..., (-j) mod n, i]
    y = np.swapaxes(k, -2, -1)
    return np.concatenate([y[..., :1], y[..., :0:-1]], axis=-1)


def _symmetric_kernel(k):
    k1 = k
    k2 = _rot90_kernel(k1)
    k3 = _rot90_kernel(k2)
    k4 = _rot90_kernel(k3)
    k5 = np.swapaxes(k1, -2, -1)
    k6 = _rot90_kernel(k5)
    k7 = _rot90_kernel(k6)
    k8 = _rot90_kernel(k7)
    return (k1 + k2 + k3 + k4 + k5 + k6 + k7 + k8) / 8.0


def _prep_k_all(kernel_np):
    """kernel [1,8,16,64,64] -> k_all [128, 2048] conv-layout packed."""
    ksym = _symmetric_kernel(kernel_np.astype(np.float64))[0]   # [8,16,64,64]
    K = np.fft.rfft2(ksym).real                                  # [8,16,64,33]
    Kc = np.transpose(K[:, :, :, :32], (0, 1, 3, 2)).copy()      # [i,j,c,kx]
    Kc[:, :, :, 32] = 0.0                                        # kx nyquist
    k_all = np.zeros((128, 2048), dtype=np.float32)
    for i in range(8):
        h, im = i // 4, i % 4
        for j in range(16):
            k_all[im * 32:(im + 1) * 32, j * 128 + h * 64: j * 128 + h * 64 + 64] = Kc[i, j]
    return k_all


# ---------------------------------------------------------------------------
# device program
# ---------------------------------------------------------------------------

def _bcast(ap, n, axis_pos=1):
    """Insert a zero-step broadcast dim of size n into an AP (after partition dim)."""
    dims = list(ap.ap)
    dims.insert(axis_pos, [0, n])
    return AP(ap.tensor, ap.offset, dims)


def _view(ap, offset_elems, dims):
    """Raw AP view on the same tensor: explicit offset (elems) + [step, count] dims."""
    return AP(ap.tensor, ap.offset + offset_elems, dims)


def build_program(reps=1, ablate=(), cross_bf16=False, gps_subs=False,
                  gps_conv=False, phase_b=False, dma_split=0, gcopy_dve=False,
                  gps_prod8=0):
    """ablate: subset of {'cross','synth','conv','dma'} to skip (profiling)."""
    nc = bacc.Bacc("TRN2", target_bir_lowering=False)
    consts = _host_consts()
    BF16 = mybir.dt.bfloat16
    xdt = BF16 if cross_bf16 else F32

    f_in = nc.dram_tensor("f_in", [B_PER_CORE, C1, 64, 64], F32, kind="ExternalInput")
    k_in = nc.dram_tensor("k_all", [128, 2048], F32, kind="ExternalInput")
    # transposed output layout [b, X, ch, Y]; host returns .transpose(0,2,1,3) view
    out_sh = nc.dram_tensor("out_sh", [B_PER_CORE, 128, NCH_OUT, 128], F32,
                            kind="ExternalOutput")

    cdr = {name: nc.inline_tensor(arr, name=f"c_{name}") for name, arr in consts.items()}

    with tile.TileContext(nc) as tc:
        with (
            tc.tile_pool(name="cp", bufs=1) as cp,
            tc.tile_pool(name="fld", bufs=1) as fld,     # u_all/v_all/fr_all
            tc.tile_pool(name="wk", bufs=2) as wk,       # small working tiles
            tc.tile_pool(name="mw", bufs=1) as mwp,      # conv wide tiles
            tc.tile_pool(name="wp", bufs=2) as wp,       # cross product blocks
            tc.tile_pool(name="crp", bufs=3) as crp,     # cross output staging
            tc.tile_pool(name="pp", bufs=1, space="PSUM") as pp,
        ):
            # ---- load constants ----
            cs = {}
            for name, arr in consts.items():
                t = cp.tile(list(arr.shape), F32, tag=f"c_{name}", name=f"cs_{name}")
                nc.sync.dma_start(out=t[:], in_=cdr[name][:])
                cs[name] = t
            k_sb = cp.tile([128, 2048], F32, tag="k_sb")
            nc.sync.dma_start(out=k_sb[:], in_=k_in[:])

            u_all = fld.tile([128, 16 * 256], xdt, tag="u_all")
            v_all = fld.tile([128, 16 * 256], xdt, tag="v_all")
            fr_all = fld.tile([128, 8 * 256], F32, tag="fr_all")

            dma_tick = [0]

            def out_dma(out_ap, in_ap, ring=None):
                # dma_split = modulus M: every M-th output DMA issues on the ACT ring
                if ring is not None:
                    eng = ring
                elif dma_split and dma_tick[0] % dma_split == dma_split - 1:
                    eng = nc.scalar
                else:
                    eng = nc.sync
                dma_tick[0] += 1
                eng.dma_start(out=out_ap, in_=in_ap)

            prod_tick = [0]

            def prod_eng():
                i = prod_tick[0] % 8
                prod_tick[0] += 1
                return nc.gpsimd if i < gps_prod8 else nc.vector

            def emit_cross_block(gI, gJ, b):
                """cross products for channel groups gI x gJ, one sample."""
                # late blocks drain on the otherwise-idle ACT ring
                ring = nc.scalar if (b == B_PER_CORE - 1 and gJ >= 2) else None
                W1 = wp.tile([128, 2048], xdt, tag="W1", name="W1")
                for ai in range(4):
                    a = 4 * gI + ai
                    in0 = _view(u_all[:], a * 256 + b * 128,
                                [u_all[:].ap[0], [0, 4], [1, 128]])
                    in1 = _view(v_all[:], gJ * 1024 + b * 128,
                                [v_all[:].ap[0], [256, 4], [1, 128]])
                    out = W1[:, ai * 512:(ai + 1) * 512].rearrange(
                        "p (cb f) -> p cb f", cb=4)
                    prod_eng().tensor_mul(out, in0, in1)
                if gI != gJ:
                    W2 = wp.tile([128, 2048], xdt, tag="W2", name="W2")
                    for bjl in range(4):
                        bj = 4 * gJ + bjl
                        in0 = _view(u_all[:], bj * 256 + b * 128,
                                    [u_all[:].ap[0], [0, 4], [1, 128]])
                        in1 = _view(v_all[:], gI * 1024 + b * 128,
                                    [v_all[:].ap[0], [256, 4], [1, 128]])
                        out = W2[:, bjl * 512:(bjl + 1) * 512].rearrange(
                            "p (ca f) -> p ca f", ca=4)
                        prod_eng().tensor_mul(out, in0, in1)
                    for ai in range(4):
                        a = 4 * gI + ai
                        cr = crp.tile([128, 512], F32, tag="cr", name="cr")
                        in0 = W1[:, ai * 512:(ai + 1) * 512].rearrange(
                            "p (cb f) -> p cb f", cb=4)
                        in1 = _view(W2[:], ai * 128,
                                    [W2[:].ap[0], [512, 4], [1, 128]])
                        sub_eng = nc.gpsimd if gps_subs else nc.vector
                        sub_eng.tensor_sub(
                            cr[:].rearrange("p (cb f) -> p cb f", cb=4), in0, in1)
                        pch = 8 + _PAIR_IDX[(a, 4 * gJ)]
                        if 'dma' not in ablate:
                            out_dma(out_sh[b, :, pch:pch + 4, :],
                                    cr[:].rearrange("x (c y) -> x c y", c=4), ring=ring)
                else:
                    for ai in range(3):
                        a = 4 * gI + ai
                        cnt = 3 - ai
                        cr = crp.tile([128, 512], F32, tag="cr", name="cr")
                        in0 = _view(W1[:], ai * 512 + (ai + 1) * 128,
                                    [W1[:].ap[0], [128, cnt], [1, 128]])
                        in1 = _view(W1[:], (ai + 1) * 512 + ai * 128,
                                    [W1[:].ap[0], [512, cnt], [1, 128]])
                        sub_eng = nc.gpsimd if gps_subs else nc.vector
                        sub_eng.tensor_sub(
                            cr[:, 0:cnt * 128].rearrange(
                                "p (cb f) -> p cb f", cb=cnt), in0, in1)
                        pch = 8 + _PAIR_IDX[(a, a + 1)]
                        if 'dma' not in ablate:
                            out_dma(out_sh[b, :, pch:pch + cnt, :],
                                    cr[:, 0:cnt * 128].rearrange("x (c y) -> x c y", c=cnt),
                                    ring=ring)

            def emit_stage1(b, st):
                A_ch = []
                T1s = []
                for ip in range(4):
                    fsb = wk.tile([64, 128], F32, tag="fsb", name="fsb")
                    nc.sync.dma_start(
                        out=fsb[:].rearrange("x (i y) -> x i y", i=2),
                        in_=f_in[b, 2 * ip:2 * ip + 2].rearrange("i x y -> x i y"))
                    psA = pp.tile([128, 192], F32, tag="bankA", bufs=2, name="psA")
                    nc.tensor.matmul(psA[:], fsb[:], cs["ExF"][:], start=True, stop=True)
                    for iloc in range(2):
                        a_t = wk.tile([64, 192], F32, tag=f"ach{2*ip+iloc}",
                                      name=f"ach{2*ip+iloc}")
                        nc.vector.tensor_copy(a_t[:], psA[iloc * 64:(iloc + 1) * 64, :])
                        A_ch.append(a_t)
                    # fr path: T1 = [f_i^T Rx^T | f_{i+1}^T Rx^T]  ([y, X] per channel)
                    psT1 = pp.tile([128, 128], F32, tag="bankA", bufs=2, name="psT1")
                    nc.tensor.matmul(psT1[:], fsb[:], cs["RxT"][:], start=True, stop=True)
                    t1sb = wk.tile([128, 128], F32, tag=f"t1sb{ip}", name=f"t1sb{ip}")
                    nc.scalar.copy(out=t1sb[:], in_=psT1[:])
                    T1s.append(t1sb)
                st['A_ch'] = A_ch
                st['T1s'] = T1s

            def emit_stage2(b, st):
                A_ch = st['A_ch']
                # out free = [F_R(kx64) | F_I(kx64)] per tile
                psFcv = [pp.tile([128, 128], F32, tag=f"bankF{4+h}", name=f"psFcv{h}")
                         for h in range(2)]
                EyC, EyS = cs["EyCT"], cs["EyST"]
                for i in range(8):
                    A_RI = A_ch[i][:, 0:128]     # [A_R | A_I]
                    A_IS = A_ch[i][:, 64:192]    # [A_I | -A_R]
                    h, im = i // 4, i % 4
                    sl = slice(im * 32, (im + 1) * 32)
                    tp = (0, im * 32)
                    nc.tensor.matmul(psFcv[h][sl, :], EyC[:], A_RI, start=True, stop=False,
                                     tile_position=tp)
                    nc.tensor.matmul(psFcv[h][sl, :], EyS[:], A_IS, start=False, stop=True,
                                     tile_position=tp)

                Fcv = wk.tile([128, 256], F32, tag="Fcv", name="Fcv")
                for h in range(2):
                    nc.vector.tensor_copy(Fcv[:, h * 64:(h + 1) * 64], psFcv[h][:, 0:64])
                    nc.vector.tensor_copy(Fcv[:, 128 + h * 64:128 + (h + 1) * 64],
                                          psFcv[h][:, 64:128])
                st['Fcv'] = Fcv

            def emit_conv(b, st):
                Fcv = st['Fcv']
                Mw = []
                for RI in range(2):
                    m_t = mwp.tile([128, 2048], F32, tag=f"mw{RI}", name=f"mw{RI}")
                    in0 = _bcast(Fcv[:, RI * 128:(RI + 1) * 128], 16)
                    conv_eng = nc.gpsimd if gps_conv else nc.vector
                    conv_eng.tensor_mul(
                        m_t[:].rearrange("p (j f) -> p j f", j=16),
                        in0,
                        k_sb[:].rearrange("p (j f) -> p j f", j=16))
                    Mw.append(m_t)

                BuR = wk.tile([32, 1024], F32, tag="BuR", name="BuR")
                BuI = wk.tile([32, 1024], F32, tag="BuI", name="BuI")
                BvR = wk.tile([32, 1024], F32, tag="BvR", name="BvR")
                BvI = wk.tile([32, 1024], F32, tag="BvI", name="BvI")
                tsg = cs["tsg"]
                for RI in range(2):
                    for jh in range(2):
                        ps_acv = pp.tile([32, 512], F32, tag="bankA", bufs=2, name="ps_acv")
                        for h in range(2):
                            rhs = _view(Mw[RI][:], jh * 1024 + h * 64,
                                        [Mw[RI][:].ap[0], [128, 8], [1, 64]])
                            nc.tensor.matmul(ps_acv[:], cs["S_sel"][:], rhs,
                                             start=(h == 0), stop=(h == 1))
                        osl = slice(jh * 512, (jh + 1) * 512)
                        if RI == 0:  # A_R -> imaginary parts of Bu/Bv
                            nc.vector.tensor_mul(BuI[:, osl], ps_acv[:], tsg[:, 512:1024])
                            nc.vector.tensor_mul(BvI[:, osl], ps_acv[:], tsg[:, 1536:2048])
                        else:        # A_I -> real parts (negated multipliers)
                            nc.vector.tensor_mul(BuR[:, osl], ps_acv[:], tsg[:, 0:512])
                            nc.vector.tensor_mul(BvR[:, osl], ps_acv[:], tsg[:, 1024:1536])
                st['B'] = (BuR, BuI, BvR, BvI)

            def emit_synth(b, st):
                BuR, BuI, BvR, BvI = st['B']

                # fr direct: fr_i = (T1_i)^T @ Cy^T via one matmul per channel
                for i in range(8):
                    ip, iloc = i // 2, i % 2
                    t1 = st['T1s'][ip][iloc * 64:(iloc + 1) * 64, :]
                    psUf = pp.tile([128, 128], F32, tag=f"bankF{2 + i % 2}", name="psUf")
                    nc.tensor.matmul(psUf[:], t1,
                                     cs["CyT"][iloc * 64:(iloc + 1) * 64, :],
                                     start=True, stop=True)
                    nc.scalar.copy(out=fr_all[:, i * 256 + b * 128:i * 256 + (b + 1) * 128],
                                   in_=psUf[:])
                if 'dma' not in ablate:
                    frv = _view(fr_all[:], b * 128,
                                [fr_all[:].ap[0], [256, 8], [1, 128]])
                    out_dma(out_sh[b, :, 0:8, :], frv)

                def bu_slices(cpair):
                    csl = slice(cpair * 128, (cpair + 1) * 128)
                    return (BuR[:, csl], BuI[:, csl])

                def bv_slices(cpair):
                    csl = slice(cpair * 128, (cpair + 1) * 128)
                    return (BvR[:, csl], BvI[:, csl])

                fields = [
                    (bu_slices, u_all, 16, False),
                    (bv_slices, v_all, 16, False),
                ]
                for get_sl, dest, nch, is_fr in fields:
                    for cpair in range(nch // 2):
                        BRs, BIs = get_sl(cpair)
                        psG = pp.tile([128, 256], F32, tag=f"bankF{cpair % 2}", name="psG")
                        nc.tensor.matmul(psG[:], BRs, cs["QF1"][:], start=True, stop=False)
                        nc.tensor.matmul(psG[:], BIs, cs["QF2"][:], start=False, stop=True)
                        # stacked [G_R ; G_I] x 2 channels -> one K=128 N=256 matmul
                        G_stk = wk.tile([128, 256], F32, tag="G_stk", name="G_stk")
                        for cl in range(2):
                            if gcopy_dve and cl == 1:
                                nc.vector.tensor_copy(G_stk[0:64, cl * 128:(cl + 1) * 128],
                                                      psG[cl * 64:(cl + 1) * 64, 0:128])
                                nc.vector.tensor_copy(G_stk[64:128, cl * 128:(cl + 1) * 128],
                                                      psG[cl * 64:(cl + 1) * 64, 128:256])
                            else:
                                nc.scalar.copy(out=G_stk[0:64, cl * 128:(cl + 1) * 128],
                                               in_=psG[cl * 64:(cl + 1) * 64, 0:128])
                                nc.scalar.copy(out=G_stk[64:128, cl * 128:(cl + 1) * 128],
                                               in_=psG[cl * 64:(cl + 1) * 64, 128:256])
                        psU = pp.tile([128, 256], F32, tag=f"bankF{2 + cpair % 2}", name="psU")
                        nc.tensor.matmul(psU[:], cs["PRTPnIT"][:], G_stk[:],
                                         start=True, stop=True)
                        dsl = _view(dest[:], (2 * cpair) * 256 + b * 128,
                                    [dest[:].ap[0], [256, 2], [1, 128]])
                        if is_fr:
                            nc.scalar.copy(out=dsl, in_=psU[:].rearrange(
                                "p (c y) -> p c y", c=2))
                        else:
                            nc.vector.tensor_copy(dsl, psU[:].rearrange(
                                "p (c y) -> p c y", c=2))


            def emit_cross(b, st):
                for gI in range(4):
                    for gJ in range(gI, 4):
                        emit_cross_block(gI, gJ, b)

            for rep in range(reps):
                st = {b: {} for b in range(B_PER_CORE)}
                for b in range(B_PER_CORE):
                    emit_stage1(b, st[b])
                for b in range(B_PER_CORE):
                    emit_stage2(b, st[b])
                if 'conv' in ablate:
                    continue
                for b in range(B_PER_CORE):
                    emit_conv(b, st[b])
                if 'synth' in ablate:
                    continue
                for b in range(B_PER_CORE):
                    emit_synth(b, st[b])
                if 'cross' in ablate:
                    continue
                for b in range(B_PER_CORE):
                    emit_cross(b, st[b])
    nc.compile()
    return nc


# ---------------------------------------------------------------------------
# entry point
# ---------------------------------------------------------------------------

_PROGRAM = {}


def _get_program(reps=1, ablate=(), cross_bf16=None, **kw):
    global _PROGRAM
    import os
    if cross_bf16 is None:
        cross_bf16 = bool(os.environ.get("KBF16"))
    if 'gps_subs' not in kw:
        kw['gps_subs'] = os.environ.get("KGPS", "1") == "1"
    if 'gps_prod8' not in kw:
        kw['gps_prod8'] = int(os.environ.get("KGPSP", "4"))
    if 'gps_conv' not in kw:
        kw['gps_conv'] = os.environ.get("KGPSC", "1") == "1"
    if 'phase_b' not in kw and os.environ.get("KPHB"):
        kw['phase_b'] = True
    key = (reps, tuple(sorted(ablate)), cross_bf16, tuple(sorted(kw.items())))
    if key not in _PROGRAM:
        _PROGRAM[key] = build_program(reps, ablate=ablate, cross_bf16=cross_bf16, **kw)
    return _PROGRAM[key]


LAST_EXEC_NS = None
LAST_RESULT = None


def kernel(f, kernel):
    global LAST_EXEC_NS, LAST_RESULT
    f = np.ascontiguousarray(f, dtype=np.float32)
    k_all = _prep_k_all(np.asarray(kernel))
    nc = _get_program()
    in_maps = [
        {"f_in": f[2 * c:2 * c + 2], "k_all": k_all} for c in range(N_CORES)
    ]
    import os
    trace = bool(os.environ.get("KERNEL_TRACE"))
    res = run_bass_kernel_spmd(nc, in_maps, list(range(N_CORES)), trace=trace)
    LAST_RESULT = res
    if res.exec_time_ns is not None:
        LAST_EXEC_NS = res.exec_time_ns
    out = np.concatenate([res.results[c]["out_sh"] for c in range(N_CORES)], axis=0)
    # device layout is [b, X, ch, Y]; return the [b, ch, X, Y] view
    return out.transpose(0, 2, 1, 3)



# revision 9
# speedup vs baseline: 2.0598x; 2.0598x over previous
"""Trainium2 Bass kernel for nn_EquivariantLayer — bf16 redesign.

Data-parallel over batch (2 samples/core x 8 cores). All DFTs are matmuls
on the TensorEngine in bf16 (1 cyc/row vs 4 for f32). Cross products on
DVE/Pool in bf16 (2x DVE mode). Output staged per-sample in SBUF (bf16)
and stored with 3 chunk-DMAs per sample over the 3 DMA queues
(SP / Activation / Pool). Host casts bf16 -> f32.

Per sample:
  fsb    = cast-load f (Pool SWDGE, f32->bf16)      [64, (i8,y64)]
  psA    = fsb^T @ ExF    (x-DFT)                   [128, (RI,kx) 192]
  psT1   = fsb^T @ RxT    (fr row transform)        [128, X 128]
  Fcv    = y-DFT (EyC/EyS accumulate)               [128, (h,RI,kx) 256]
  Mw[RI] = Fcv (x) k_sb   (conv products)           [128, 2048]
  ps_acv = S_sel @ Mw     (i-reduction)             [32, 512] x4
  Bu/Bv  = acv (x) tsg    (uncurl multipliers)      [32, 1024] x4
  psG    = B @ QF1 + B_I @ QF2  (ky-inverse)        [128=(ch2,kx), (RI,Y) 256]
  Gsb    = drain psG (1 copy)
  psU    = PRT64 @ G_R - PIT64 @ G_I per ch (kx-inverse, accumulated
           into column blocks, 4 ch per PSUM bank)  [128, 512]
  u_s/v_s fields bf16; fr direct path via CyT
  W      = u_a * v_b all 16x16 ordered products     [128, 32768] bf16
  subs   -> staging tiles st0/st1/st2 (bf16), ch-grouped
  DMA    st* -> out_sh[b] on SP/ACT/POOL queues
"""
import sys
import numpy as np
import ml_dtypes

if '/opt/trn_rl_repo' not in sys.path:
    sys.path.insert(0, '/opt/trn_rl_repo')

import concourse.bass as bass
from concourse import bacc
import concourse.mybir as mybir
import concourse.tile as tile
from concourse.bass import AP
from concourse.bass_utils import run_bass_kernel_spmd

F32 = mybir.dt.float32
BF16 = mybir.dt.bfloat16
N_CORES = 8
B_PER_CORE = 2
C1, C2, N1, N2 = 8, 16, 64, 128
NCH_OUT = 128

PAIR_BASE = {}
_p = 8
for _a in range(15):
    PAIR_BASE[_a] = _p
    _p += 15 - _a
assert _p == 128

CH_SPLITS = [0, 37, 62, 83, 100, 113, 128]  # a-run aligned chunk bounds


def _bf16(a):
    return np.ascontiguousarray(np.asarray(a, dtype=np.float32),
                                dtype=ml_dtypes.bfloat16)


def _host_consts():
    x = np.arange(64)
    kx = np.arange(64)
    c = np.arange(32)
    y = np.arange(64)
    X = np.arange(128)
    Y = np.arange(128)

    FRs = np.where(kx <= 32, kx, kx - 64).astype(np.float64)

    ExR = np.cos(2 * np.pi * np.outer(kx, x) / 64)
    ExI = -np.sin(2 * np.pi * np.outer(kx, x) / 64)
    ExF = np.concatenate([ExR.T, ExI.T, -ExR.T], axis=1)     # [x, 192]

    EyCT = np.cos(2 * np.pi * np.outer(c, y) / 64).T          # [y, 32]
    EyST = np.sin(2 * np.pi * np.outer(c, y) / 64).T

    S_sel = np.zeros((128, 32))
    for im in range(4):
        S_sel[im * 32 + np.arange(32), np.arange(32)] = 1.0

    den = FRs[None, :] ** 2 + c[:, None].astype(np.float64) ** 2
    den[0, 0] = 1.0
    t_u = c[:, None] / den                            # [32, 64]
    s_v = -FRs[None, :] / den
    t_rep = np.tile(t_u, (1, 8))                      # [32, 512]
    s_rep = np.tile(s_v, (1, 8))
    tsg = np.concatenate([-t_rep, t_rep, -s_rep, s_rep], axis=1)  # [32, 2048]

    w_c = np.where(c == 0, 1.0, 2.0)
    s_q = 2.0 / (128.0 * 128.0)
    QRT = (s_q * w_c[None, :] * np.cos(2 * np.pi * np.outer(Y, c) / 128)).T
    QIT = (s_q * w_c[None, :] * np.sin(2 * np.pi * np.outer(Y, c) / 128)).T
    QFRs = np.concatenate([QRT, -QIT], axis=0)        # [64, 128] K-stack
    QFIs = np.concatenate([QIT, QRT], axis=0)

    PRT = np.cos(2 * np.pi * np.outer(FRs, X) / 128)  # [64, 128]
    PIT = np.sin(2 * np.pi * np.outer(FRs, X) / 128)
    PRT[32, :] = 0.0
    PIT[32, :] = 0.0
    PRTPnIT = np.concatenate([PRT, -PIT], axis=0)     # [128, 128] K-stack

    # fr direct path
    EyRm = np.cos(2 * np.pi * np.outer(c, y) / 64)
    EyIm = -np.sin(2 * np.pi * np.outer(c, y) / 64)
    QRm = s_q * w_c[None, :] * np.cos(2 * np.pi * np.outer(Y, c) / 128)
    QIm = s_q * w_c[None, :] * np.sin(2 * np.pi * np.outer(Y, c) / 128)
    Rx = PRT.T @ ExR - PIT.T @ ExI                    # [128, 64]
    Cy = QRm @ EyRm - QIm @ EyIm                      # [128, 64]
    RxT = Rx.T                                        # [64, 128]
    CyT = np.concatenate([Cy.T, Cy.T], axis=0)        # [128, 128]

    dup = lambda m: np.concatenate([m, m], axis=0)   # both partition halves
    return dict(ExF=_bf16(ExF), EyCT=_bf16(dup(EyCT)), EyST=_bf16(dup(EyST)),
                S_sel=_bf16(S_sel), tsg=_bf16(tsg), QF1=_bf16(QF1),
                QF2=_bf16(QF2), PRT64=_bf16(dup(PRT)), nPIT64=_bf16(dup(nPIT)),
                RxT=_bf16(RxT), CyT=_bf16(CyT))


def _rot90_kernel(k):
    yk = np.swapaxes(k, -2, -1)
    return np.concatenate([yk[..., :1], yk[..., :0:-1]], axis=-1)


def _symmetric_kernel(k):
    k1 = k
    k2 = _rot90_kernel(k1)
    k3 = _rot90_kernel(k2)
    k4 = _rot90_kernel(k3)
    k5 = np.swapaxes(k1, -2, -1)
    k6 = _rot90_kernel(k5)
    k7 = _rot90_kernel(k6)
    k8 = _rot90_kernel(k7)
    return (k1 + k2 + k3 + k4 + k5 + k6 + k7 + k8) / 8.0


def _prep_k_all(kernel_np):
    """kernel [1,8,16,64,64] -> k_all [128, 2048] conv-layout, bf16."""
    ksym = _symmetric_kernel(kernel_np.astype(np.float64))[0]
    K = np.fft.rfft2(ksym).real                                 # [8,16,64,33]
    Kc = np.transpose(K[:, :, :, :32], (0, 1, 3, 2)).copy()     # [i,j,c,kx]
    Kc[:, :, :, 32] = 0.0
    k_all = np.zeros((128, 2048), dtype=np.float64)
    for i in range(8):
        h, im = i // 4, i % 4
        for j in range(16):
            k_all[im * 32:(im + 1) * 32,
                  j * 128 + h * 64: j * 128 + h * 64 + 64] = Kc[i, j]
    return _bf16(k_all)


def _bcast(ap, n, axis_pos=1):
    dims = list(ap.ap)
    dims.insert(axis_pos, [0, n])
    return AP(ap.tensor, ap.offset, dims)


def _view(ap, offset_elems, dims):
    return AP(ap.tensor, ap.offset + offset_elems, dims)


DEFAULT_CFG = dict(
    d_at='act', d_t1='act', d_fcv='act', d_acv='act', d_g='act',
    d_psu='act', d_fr='act',
    d_at0='dve', d_t10='dve', d_fcv0='dve',
    mw_eng=None, bubv_eng=None,
    tt_pool_frac=0.5,        # relative weight of Pool in the TT split
    dma_chunks=(('sp', 'act') * 3, ('act', 'sp') * 3),
)


def build_program(reps=1, **cfg_over):
    cfg = dict(DEFAULT_CFG)
    cfg.update(cfg_over)
    nc = bacc.Bacc("TRN2", target_bir_lowering=False)
    consts = _host_consts()

    f_in = nc.dram_tensor("f_in", [B_PER_CORE, C1, 64, 64], F32,
                          kind="ExternalInput")
    k_in = nc.dram_tensor("k_all", [128, 2048], BF16, kind="ExternalInput")
    out_sh = nc.dram_tensor("out_sh", [B_PER_CORE, 128, NCH_OUT, 128], BF16,
                            kind="ExternalOutput")

    cdr = {n: nc.inline_tensor(a, name=f"c_{n}") for n, a in consts.items()}

    mix_tick = [0]

    def drain(which, out_ap, in_ap):
        e = cfg[which]
        if e == 'mix':
            mix_tick[0] += 1
            e = 'dve' if mix_tick[0] % 2 else 'act'
        if e == 'dve':
            nc.vector.tensor_copy(out_ap, in_ap)
        else:
            nc.scalar.copy(out=out_ap, in_=in_ap)

    # weighted greedy balance of TT ops between DVE and Pool
    tt_state = [0.0, 0.0]       # projected ns on dve, pool

    def tt_eng(fe, pref=None):
        if pref == 'dve':
            tt_state[0] += fe * 0.521 + 60
            return nc.vector
        if pref == 'gps':
            tt_state[1] += fe * 0.833 + 25
            return nc.gpsimd
        w_pool = cfg['tt_pool_frac']
        t_d = (tt_state[0] + fe * 0.521 + 60) / max(1.0 - w_pool, 1e-6)
        t_p = (tt_state[1] + fe * 0.833 + 25) / max(w_pool, 1e-6)
        if t_p < t_d:
            tt_state[1] += fe * 0.833 + 25
            return nc.gpsimd
        tt_state[0] += fe * 0.521 + 60
        return nc.vector

    with tile.TileContext(nc) as tc:
        with (
            tc.tile_pool(name="cp", bufs=1) as cp,
            tc.tile_pool(name="wk", bufs=2) as wk,
            tc.tile_pool(name="uv", bufs=2) as uvp,
            tc.tile_pool(name="wp", bufs=1) as wp,
            tc.tile_pool(name="stp", bufs=2) as stp,
            tc.tile_pool(name="pp", bufs=1, space="PSUM") as pp,
        ):
            # ---- loads: f first (needed earliest), consts spread ----
            st = {b: {} for b in range(B_PER_CORE)}
            for b in range(B_PER_CORE):
                fsb32 = wk.tile([64, 512], F32, tag="fsb32", name="fsb32")
                nc.sync.dma_start(
                    out=fsb32[:].rearrange("x (i y) -> x i y", i=8),
                    in_=f_in[b].rearrange("i x y -> x i y"))
                fsb = wk.tile([64, 512], BF16, tag="fsb", name="fsb")
                nc.vector.tensor_copy(fsb[:], fsb32[:])
                st[b]['fsb'] = fsb

            cs = {}
            lq = [nc.sync, nc.scalar]
            order = ["ExF", "RxT", "EyCT", "EyST", "S_sel", "CyT",
                     "tsg", "QF1", "QF2", "PRT64", "nPIT64"]
            for li, name in enumerate(order):
                arr = consts[name]
                t = cp.tile(list(arr.shape), BF16, tag=f"c_{name}",
                            name=f"cs_{name}")
                lq[li % 2].dma_start(out=t[:], in_=cdr[name][:])
                cs[name] = t
            k_sb = cp.tile([128, 2048], BF16, tag="k_sb", name="k_sb")
            nc.scalar.dma_start(out=k_sb[:], in_=k_in[:])

            qmap = {'sp': nc.sync, 'act': nc.scalar, 'gps': nc.gpsimd}

            def emit_stage1(b):
                s = st[b]
                fsb = s['fsb']
                a_ts, t1s = [], []
                for ip2 in range(2):
                    psA = pp.tile([128, 384], F32, tag="bankA", bufs=2,
                                  name="psA")
                    psT1 = pp.tile([64, 512], F32, tag="bankA", bufs=2,
                                   name="psT1")
                    for ipl in range(2):
                        ip = 2 * ip2 + ipl
                        lhs = fsb[:, ip * 128:(ip + 1) * 128]
                        nc.tensor.matmul(psA[:, ipl * 192:(ipl + 1) * 192],
                                         lhs, cs["ExF"][:],
                                         start=True, stop=True)
                    for k in range(4):
                        ch = 4 * ip2 + k
                        nc.tensor.matmul(psT1[:, k * 128:(k + 1) * 128],
                                         fsb[:, ch * 64:(ch + 1) * 64],
                                         cs["RxT"][:],
                                         start=True, stop=True)
                    a_t = wk.tile([128, 384], BF16, tag=f"at{ip2}",
                                  name=f"at{ip2}")
                    drain('d_at', a_t[:], psA[:], b)
                    a_ts.append(a_t)
                    t1 = wk.tile([64, 512], BF16, tag=f"t1{ip2}",
                                 name=f"t1{ip2}")
                    drain('d_t1', t1[:], psT1[:], b)
                    t1s.append(t1)
                s['a_ts'] = a_ts
                s['t1s'] = t1s

            def emit_stage2(b):
                s = st[b]
                psF = [pp.tile([128, 128], F32, tag=f"bankF{h}",
                               name=f"psF{h}") for h in range(2)]
                for i in range(8):
                    a_t = s['a_ts'][i // 4]
                    base = ((i // 2) % 2) * 192
                    po = (i % 2) * 64
                    A_RI = a_t[po:po + 64, base:base + 128]
                    A_IS = a_t[po:po + 64, base + 64:base + 192]
                    h, im = i // 4, i % 4
                    sl = slice(im * 32, (im + 1) * 32)
                    tp = (po, im * 32)
                    nc.tensor.matmul(psF[h][sl, :],
                                     cs["EyCT"][po:po + 64, :], A_RI,
                                     start=True, stop=False, tile_position=tp)
                    nc.tensor.matmul(psF[h][sl, :],
                                     cs["EyST"][po:po + 64, :], A_IS,
                                     start=False, stop=True, tile_position=tp)
                Fcv = wk.tile([128, 256], BF16, tag="Fcv", name="Fcv")
                for h in range(2):
                    drain('d_fcv', Fcv[:, h * 128:(h + 1) * 128], psF[h][:])
                s['Fcv'] = Fcv

            def emit_conv(b):
                s = st[b]
                Fcv = s['Fcv']
                Mw = []
                for RI in range(2):
                    m_t = wp.tile([128, 2048], BF16, tag=f"mw{RI}", bufs=2,
                                  name=f"mw{RI}")
                    in0 = _view(Fcv[:], RI * 64,
                                [Fcv[:].ap[0], [0, 16], [128, 2], [1, 64]])
                    tt_eng(2048, cfg['mw_eng']).tensor_mul(
                        m_t[:].rearrange("p (j h f) -> p j h f", j=16, h=2),
                        in0,
                        k_sb[:].rearrange("p (j h f) -> p j h f", j=16, h=2))
                    Mw.append(m_t)

                Bu = wk.tile([64, 1024], BF16, tag="Bu", name="Bu")
                Bv = wk.tile([64, 1024], BF16, tag="Bv", name="Bv")
                BuR, BuI = Bu[0:32, :], Bu[32:64, :]
                BvR, BvI = Bv[0:32, :], Bv[32:64, :]
                tsg = cs["tsg"]
                for RI in range(2):
                    for jh in range(2):
                        ps_acv = pp.tile([32, 512], F32, tag="bankA", bufs=2,
                                         name="ps_acv")
                        for h in range(2):
                            rhs = _view(Mw[RI][:], jh * 1024 + h * 64,
                                        [Mw[RI][:].ap[0], [128, 8], [1, 64]])
                            nc.tensor.matmul(ps_acv[:], cs["S_sel"][:], rhs,
                                             start=(h == 0), stop=(h == 1))
                        if cfg.get('acv_direct'):
                            tt_state[0] += 2 * (512 * 1.0417 + 60)
                            beng0 = beng1 = nc.vector
                            src_ap = ps_acv[:]
                        else:
                            acv = wk.tile([32, 512], BF16, tag="acv",
                                          name="acv")
                            drain('d_acv', acv[:], ps_acv[:], b)
                            src_ap = acv[:]
                            beng0 = tt_eng(512, cfg['bubv_eng'])
                            beng1 = tt_eng(512, cfg['bubv_eng'])
                        osl = slice(jh * 512, (jh + 1) * 512)
                        if RI == 0:
                            beng0.tensor_mul(BuI[:, osl], src_ap,
                                             tsg[:, 512:1024])
                            beng1.tensor_mul(BvI[:, osl], src_ap,
                                             tsg[:, 1536:2048])
                        else:
                            beng0.tensor_mul(BuR[:, osl], src_ap,
                                             tsg[:, 0:512])
                            beng1.tensor_mul(BvR[:, osl], src_ap,
                                             tsg[:, 1024:1536])
                s['B'] = (Bu, Bv)

            def emit_staging(b):
                s = st[b]
                sts = []
                for ci in range(len(CH_SPLITS) - 1):
                    ncols = (CH_SPLITS[ci + 1] - CH_SPLITS[ci]) * 128
                    stt = stp.tile([128, ncols], BF16, tag=f"st{ci}",
                                   name=f"st{ci}")
                    if cfg.get('level', 99) < 4:
                        nc.vector.memset(stt[:], 0.0)
                    sts.append(stt)
                s['sts'] = sts

            def emit_fr(b):
                s = st[b]
                sts = s['sts']
                for ip2 in range(2):
                    psUf = pp.tile([128, 512], F32, tag=f"bankF{2 + ip2}",
                                   name="psUf")
                    for k in range(4):
                        nc.tensor.matmul(
                            psUf[:, k * 128:(k + 1) * 128],
                            s['t1s'][ip2][0:64, k * 128:(k + 1) * 128],
                            cs["CyT"][0:64, :],
                            start=True, stop=True)
                    drain('d_fr', sts[0][:, ip2 * 512:(ip2 + 1) * 512],
                          psUf[:], b)

            ps_tick = [0]

            def emit_synth(b):
                s = st[b]
                Bu, Bv = s['B']
                u_q, v_q = [], []
                for q in range(4):
                    for B_, dst_list in ((Bu, u_q), (Bv, v_q)):
                        ps_tick[0] += 1
                        gsbs = []
                        for chl in range(2):       # 2 cpairs per quad
                            cpair = q * 2 + chl
                            psG = pp.tile([128, 256], F32,
                                          tag=f"bankF{2 + chl}",
                                          name="psG")
                            for ch2 in range(2):
                                ch = cpair * 2 + ch2
                                lhsT = B_[0:64, ch * 64:(ch + 1) * 64]
                                ccol = slice(ch2 * 128, (ch2 + 1) * 128)
                                nc.tensor.matmul(
                                    psG[0:64, ccol], lhsT, cs["QFRs"],
                                    start=True, stop=True,
                                    tile_position=(0, 0))
                                nc.tensor.matmul(
                                    psG[64:128, ccol], lhsT, cs["QFIs"],
                                    start=True, stop=True,
                                    tile_position=(0, 64))
                            gsb = wk.tile([128, 256], BF16, tag=f"gsb{chl}",
                                          name=f"gsb{chl}", bufs=2)
                            drain('d_g', gsb[:], psG[:], b)
                            gsbs.append(gsb)
                        psU = pp.tile([128, 512], F32,
                                      tag=f"bankF{4 + ps_tick[0] % 2}",
                                      name="psU")
                        for chloc in range(4):
                            gsb = gsbs[chloc // 2]
                            ch2 = chloc % 2
                            nc.tensor.matmul(
                                psU[:, chloc * 128:(chloc + 1) * 128],
                                cs["PRTPnIT"],
                                gsb[:, ch2 * 128:(ch2 + 1) * 128],
                                start=True, stop=True)
                        nm = ('u' if dst_list is u_q else 'v') + f"q{q}"
                        qt = uvp.tile([128, 512], BF16, tag=nm, name=nm)
                        drain('d_psu', qt[:], psU[:], b)
                        dst_list.append(qt)
                s['u_q'] = u_q
                s['v_q'] = v_q

            def emit_cross(b):
                s = st[b]
                u_q, v_q, sts = s['u_q'], s['v_q'], s['sts']

                def st_sub(a, b0, cnt, in0, in1):
                    pch = PAIR_BASE[a] + (b0 - a - 1)
                    ci = max(i for i in range(len(CH_SPLITS) - 1)
                             if CH_SPLITS[i] <= pch)
                    assert pch + cnt <= CH_SPLITS[ci + 1], (a, b0, cnt)
                    out = sts[ci][:, (pch - CH_SPLITS[ci]) * 128:
                                  (pch - CH_SPLITS[ci] + cnt) * 128]
                    tt_eng(cnt * 128).tensor_sub(
                        out.rearrange("p (c y) -> p c y", c=cnt), in0, in1)

                for gI in range(4):
                    for gJ in range(gI, 4):
                        W1 = wp.tile([128, 2048], BF16, tag="W1", bufs=3,
                                     name="W1")
                        for ai in range(4):
                            out = W1[:, ai * 512:(ai + 1) * 512].rearrange(
                                "p (c y) -> p c y", c=4)
                            in0 = _bcast(
                                u_q[gI][:, ai * 128:(ai + 1) * 128], 4)
                            in1 = v_q[gJ][:].rearrange(
                                "p (c y) -> p c y", c=4)
                            tt_eng(512).tensor_mul(out, in0, in1)
                        if gI != gJ:
                            W2 = wp.tile([128, 2048], BF16, tag="W2", bufs=3,
                                         name="W2")
                            for bj in range(4):
                                out = W2[:, bj * 512:(bj + 1) * 512].rearrange(
                                    "p (c y) -> p c y", c=4)
                                in0 = _bcast(
                                    u_q[gJ][:, bj * 128:(bj + 1) * 128], 4)
                                in1 = v_q[gI][:].rearrange(
                                    "p (c y) -> p c y", c=4)
                                tt_eng(512).tensor_mul(out, in0, in1)
                            for ai in range(4):
                                a = 4 * gI + ai
                                in0 = _view(W1[:], ai * 512,
                                            [W1[:].ap[0], [128, 4], [1, 128]])
                                in1 = _view(W2[:], ai * 128,
                                            [W2[:].ap[0], [512, 4], [1, 128]])
                                st_sub(a, 4 * gJ, 4, in0, in1)
                        else:
                            for ai in range(3):
                                a = 4 * gI + ai
                                cnt = 3 - ai
                                in0 = _view(W1[:], ai * 512 + (ai + 1) * 128,
                                            [W1[:].ap[0], [128, cnt],
                                             [1, 128]])
                                in1 = _view(W1[:], (ai + 1) * 512 + ai * 128,
                                            [W1[:].ap[0], [512, cnt],
                                             [1, 128]])
                                st_sub(a, a + 1, cnt, in0, in1)

            def emit_dma(b):
                s = st[b]
                for ci in range(len(CH_SPLITS) - 1):
                    c0, c1 = CH_SPLITS[ci], CH_SPLITS[ci + 1]
                    eng = qmap[cfg['dma_chunks'][b][ci]]
                    eng.dma_start(
                        out=out_sh[b, :, c0:c1, :],
                        in_=s['sts'][ci][:].rearrange("p (c y) -> p c y",
                                                      c=c1 - c0))

            lvl = cfg.get('level', 99)
            phases = [emit_staging, emit_stage1, emit_stage2]
            if lvl >= 1:
                phases.append(emit_conv)
            if lvl >= 2:
                phases.append(emit_fr)
            if lvl >= 3:
                phases.append(emit_synth)
            for rep in range(reps):
                for ph in phases:
                    for b in range(B_PER_CORE):
                        ph(b)
                for b in range(B_PER_CORE):
                    if lvl >= 4:
                        emit_cross(b)
                    emit_dma(b)
    nc.compile()
    return nc


_PROGRAM = {}


def _get_program(reps=1, **kw):
    key = (reps, tuple(sorted(kw.items())))
    if key not in _PROGRAM:
        _PROGRAM[key] = build_program(reps, **kw)
    return _PROGRAM[key]


LAST_EXEC_NS = None
LAST_RESULT = None


def kernel(f, kernel):
    global LAST_EXEC_NS, LAST_RESULT
    import os
    f = np.ascontiguousarray(f, dtype=np.float32)
    k_all = _prep_k_all(np.asarray(kernel))
    nc = _get_program()
    in_maps = [
        {"f_in": f[2 * c:2 * c + 2], "k_all": k_all} for c in range(N_CORES)
    ]
    trace = bool(os.environ.get("KERNEL_TRACE"))
    res = run_bass_kernel_spmd(nc, in_maps, list(range(N_CORES)), trace=trace)
    LAST_RESULT = res
    if res.exec_time_ns is not None:
        LAST_EXEC_NS = res.exec_time_ns
    out = np.concatenate(
        [np.asarray(res.results[c]["out_sh"]).astype(np.float32)
         for c in range(N_CORES)], axis=0)
    return out.transpose(0, 2, 1, 3)


# revision 11
# speedup vs baseline: 2.1464x; 1.0421x over previous
"""Trainium2 Bass kernel for nn_EquivariantLayer — bf16 redesign.

Data-parallel over batch (2 samples/core x 8 cores). All DFTs are matmuls
on the TensorEngine in bf16 (1 cyc/row vs 4 for f32). Cross products on
DVE/Pool in bf16 (2x DVE mode). Output staged per-sample in SBUF (bf16)
and stored with 3 chunk-DMAs per sample over the 3 DMA queues
(SP / Activation / Pool). Host casts bf16 -> f32.

Per sample:
  fsb    = cast-load f (Pool SWDGE, f32->bf16)      [64, (i8,y64)]
  psA    = fsb^T @ ExF    (x-DFT)                   [128, (RI,kx) 192]
  psT1   = fsb^T @ RxT    (fr row transform)        [128, X 128]
  Fcv    = y-DFT (EyC/EyS accumulate)               [128, (h,RI,kx) 256]
  Mw[RI] = Fcv (x) k_sb   (conv products)           [128, 2048]
  ps_acv = S_sel @ Mw     (i-reduction)             [32, 512] x4
  Bu/Bv  = acv (x) tsg    (uncurl multipliers)      [32, 1024] x4
  psG    = B @ QF1 + B_I @ QF2  (ky-inverse)        [128=(ch2,kx), (RI,Y) 256]
  Gsb    = drain psG (1 copy)
  psU    = PRT64 @ G_R - PIT64 @ G_I per ch (kx-inverse, accumulated
           into column blocks, 4 ch per PSUM bank)  [128, 512]
  u_s/v_s fields bf16; fr direct path via CyT
  W      = u_a * v_b all 16x16 ordered products     [128, 32768] bf16
  subs   -> staging tiles st0/st1/st2 (bf16), ch-grouped
  DMA    st* -> out_sh[b] on SP/ACT/POOL queues
"""
import sys
import numpy as np
import ml_dtypes

if '/opt/trn_rl_repo' not in sys.path:
    sys.path.insert(0, '/opt/trn_rl_repo')

import concourse.bass as bass
from concourse import bacc
import concourse.mybir as mybir
import concourse.tile as tile
from concourse.bass import AP
from concourse.bass_utils import run_bass_kernel_spmd

F32 = mybir.dt.float32
BF16 = mybir.dt.bfloat16
N_CORES = 8
B_PER_CORE = 2
C1, C2, N1, N2 = 8, 16, 64, 128
NCH_OUT = 128

PAIR_BASE = {}
_p = 8
for _a in range(15):
    PAIR_BASE[_a] = _p
    _p += 15 - _a
assert _p == 128

CH_SPLITS = [0, 37, 62, 83, 100, 113, 128]  # a-run aligned chunk bounds


def _bf16(a):
    return np.ascontiguousarray(np.asarray(a, dtype=np.float32),
                                dtype=ml_dtypes.bfloat16)


def _host_consts():
    x = np.arange(64)
    kx = np.arange(64)
    c = np.arange(32)
    y = np.arange(64)
    X = np.arange(128)
    Y = np.arange(128)

    FRs = np.where(kx <= 32, kx, kx - 64).astype(np.float64)

    ExR = np.cos(2 * np.pi * np.outer(kx, x) / 64)
    ExI = -np.sin(2 * np.pi * np.outer(kx, x) / 64)
    ExF = np.concatenate([ExR.T, ExI.T, -ExR.T], axis=1)     # [x, 192]

    EyCT = np.cos(2 * np.pi * np.outer(c, y) / 64).T          # [y, 32]
    EyST = np.sin(2 * np.pi * np.outer(c, y) / 64).T

    S_sel = np.zeros((128, 32))
    for im in range(4):
        S_sel[im * 32 + np.arange(32), np.arange(32)] = 1.0

    den = FRs[None, :] ** 2 + c[:, None].astype(np.float64) ** 2
    den[0, 0] = 1.0
    t_u = c[:, None] / den                            # [32, 64]
    s_v = -FRs[None, :] / den
    t_rep = np.tile(t_u, (1, 8))                      # [32, 512]
    s_rep = np.tile(s_v, (1, 8))
    tsg = np.concatenate([-t_rep, t_rep, -s_rep, s_rep], axis=1)  # [32, 2048]

    w_c = np.where(c == 0, 1.0, 2.0)
    s_q = 2.0 / (128.0 * 128.0)
    QRT = (s_q * w_c[None, :] * np.cos(2 * np.pi * np.outer(Y, c) / 128)).T
    QIT = (s_q * w_c[None, :] * np.sin(2 * np.pi * np.outer(Y, c) / 128)).T
    QFRs = np.concatenate([QRT, -QIT], axis=0)        # [64, 128] K-stack
    QFIs = np.concatenate([QIT, QRT], axis=0)

    PRT = np.cos(2 * np.pi * np.outer(FRs, X) / 128)  # [64, 128]
    PIT = np.sin(2 * np.pi * np.outer(FRs, X) / 128)
    PRT[32, :] = 0.0
    PIT[32, :] = 0.0
    PRTPnIT = np.concatenate([PRT, -PIT], axis=0)     # [128, 128] K-stack

    # fr direct path
    EyRm = np.cos(2 * np.pi * np.outer(c, y) / 64)
    EyIm = -np.sin(2 * np.pi * np.outer(c, y) / 64)
    QRm = s_q * w_c[None, :] * np.cos(2 * np.pi * np.outer(Y, c) / 128)
    QIm = s_q * w_c[None, :] * np.sin(2 * np.pi * np.outer(Y, c) / 128)
    Rx = PRT.T @ ExR - PIT.T @ ExI                    # [128, 64]
    Cy = QRm @ EyRm - QIm @ EyIm                      # [128, 64]
    RxT = Rx.T                                        # [64, 128]
    CyT = np.concatenate([Cy.T, Cy.T], axis=0)        # [128, 128]

    dup = lambda m: np.concatenate([m, m], axis=0)   # both partition halves
    return dict(ExF=_bf16(ExF), EyCT=_bf16(dup(EyCT)), EyST=_bf16(dup(EyST)),
                S_sel=_bf16(S_sel), tsg=_bf16(tsg), QF1=_bf16(QF1),
                QF2=_bf16(QF2), PRT64=_bf16(dup(PRT)), nPIT64=_bf16(dup(nPIT)),
                RxT=_bf16(RxT), CyT=_bf16(CyT))


def _rot90_kernel(k):
    yk = np.swapaxes(k, -2, -1)
    return np.concatenate([yk[..., :1], yk[..., :0:-1]], axis=-1)


def _symmetric_kernel(k):
    k1 = k
    k2 = _rot90_kernel(k1)
    k3 = _rot90_kernel(k2)
    k4 = _rot90_kernel(k3)
    k5 = np.swapaxes(k1, -2, -1)
    k6 = _rot90_kernel(k5)
    k7 = _rot90_kernel(k6)
    k8 = _rot90_kernel(k7)
    return (k1 + k2 + k3 + k4 + k5 + k6 + k7 + k8) / 8.0


def _prep_k_all(kernel_np):
    """kernel [1,8,16,64,64] -> k_all [128, 2048] conv-layout, bf16."""
    ksym = _symmetric_kernel(kernel_np.astype(np.float64))[0]
    K = np.fft.rfft2(ksym).real                                 # [8,16,64,33]
    Kc = np.transpose(K[:, :, :, :32], (0, 1, 3, 2)).copy()     # [i,j,c,kx]
    Kc[:, :, :, 32] = 0.0
    k_all = np.zeros((128, 2048), dtype=np.float64)
    for i in range(8):
        h, im = i // 4, i % 4
        for j in range(16):
            k_all[im * 32:(im + 1) * 32,
                  j * 128 + h * 64: j * 128 + h * 64 + 64] = Kc[i, j]
    return _bf16(k_all)


def _bcast(ap, n, axis_pos=1):
    dims = list(ap.ap)
    dims.insert(axis_pos, [0, n])
    return AP(ap.tensor, ap.offset, dims)


def _view(ap, offset_elems, dims):
    return AP(ap.tensor, ap.offset + offset_elems, dims)


DEFAULT_CFG = dict(
    d_at='act', d_t1='act', d_fcv='act', d_acv='act', d_g='act',
    d_psu='act', d_fr='act',
    d_at0='dve', d_t10='dve', d_fcv0='dve',
    acv_direct=True,
    mw_eng=None, bubv_eng=None,
    tt_pool_frac=0.5,        # relative weight of Pool in the TT split
    dma_chunks=(('sp', 'act') * 3, ('act', 'sp') * 3),
)


def build_program(reps=1, **cfg_over):
    cfg = dict(DEFAULT_CFG)
    cfg.update(cfg_over)
    nc = bacc.Bacc("TRN2", target_bir_lowering=False)
    consts = _host_consts()

    f_in = nc.dram_tensor("f_in", [B_PER_CORE, C1, 64, 64], F32,
                          kind="ExternalInput")
    k_in = nc.dram_tensor("k_all", [128, 2048], BF16, kind="ExternalInput")
    out_sh = nc.dram_tensor("out_sh", [B_PER_CORE, 128, NCH_OUT, 128], BF16,
                            kind="ExternalOutput")

    cdr = {n: nc.inline_tensor(a, name=f"c_{n}") for n, a in consts.items()}

    mix_tick = [0]

    def drain(which, out_ap, in_ap):
        e = cfg[which]
        if e == 'mix':
            mix_tick[0] += 1
            e = 'dve' if mix_tick[0] % 2 else 'act'
        if e == 'dve':
            nc.vector.tensor_copy(out_ap, in_ap)
        else:
            nc.scalar.copy(out=out_ap, in_=in_ap)

    # weighted greedy balance of TT ops between DVE and Pool
    tt_state = [0.0, 0.0]       # projected ns on dve, pool

    def tt_eng(fe, pref=None):
        if pref == 'dve':
            tt_state[0] += fe * 0.521 + 60
            return nc.vector
        if pref == 'gps':
            tt_state[1] += fe * 0.833 + 25
            return nc.gpsimd
        w_pool = cfg['tt_pool_frac']
        t_d = (tt_state[0] + fe * 0.521 + 60) / max(1.0 - w_pool, 1e-6)
        t_p = (tt_state[1] + fe * 0.833 + 25) / max(w_pool, 1e-6)
        if t_p < t_d:
            tt_state[1] += fe * 0.833 + 25
            return nc.gpsimd
        tt_state[0] += fe * 0.521 + 60
        return nc.vector

    with tile.TileContext(nc) as tc:
        with (
            tc.tile_pool(name="cp", bufs=1) as cp,
            tc.tile_pool(name="wk", bufs=2) as wk,
            tc.tile_pool(name="uv", bufs=2) as uvp,
            tc.tile_pool(name="wp", bufs=1) as wp,
            tc.tile_pool(name="stp", bufs=2) as stp,
            tc.tile_pool(name="pp", bufs=1, space="PSUM") as pp,
        ):
            # ---- loads: f first (needed earliest), consts spread ----
            st = {b: {} for b in range(B_PER_CORE)}
            for b in range(B_PER_CORE):
                fsb32 = wk.tile([64, 512], F32, tag="fsb32", name="fsb32")
                nc.sync.dma_start(
                    out=fsb32[:].rearrange("x (i y) -> x i y", i=8),
                    in_=f_in[b].rearrange("i x y -> x i y"))
                fsb = wk.tile([64, 512], BF16, tag="fsb", name="fsb")
                nc.vector.tensor_copy(fsb[:], fsb32[:])
                st[b]['fsb'] = fsb

            cs = {}
            lq = [nc.sync, nc.scalar]
            order = ["ExF", "RxT", "EyCT", "EyST", "S_sel", "CyT",
                     "tsg", "QF1", "QF2", "PRT64", "nPIT64"]
            for li, name in enumerate(order):
                arr = consts[name]
                t = cp.tile(list(arr.shape), BF16, tag=f"c_{name}",
                            name=f"cs_{name}")
                lq[li % 2].dma_start(out=t[:], in_=cdr[name][:])
                cs[name] = t
            k_sb = cp.tile([128, 2048], BF16, tag="k_sb", name="k_sb")
            nc.scalar.dma_start(out=k_sb[:], in_=k_in[:])

            qmap = {'sp': nc.sync, 'act': nc.scalar, 'gps': nc.gpsimd}

            def emit_stage1(b):
                s = st[b]
                fsb = s['fsb']
                a_ts, t1s = [], []
                for ip2 in range(2):
                    psA = pp.tile([128, 384], F32, tag="bankA", bufs=2,
                                  name="psA")
                    psT1 = pp.tile([64, 512], F32, tag="bankA", bufs=2,
                                   name="psT1")
                    for ipl in range(2):
                        ip = 2 * ip2 + ipl
                        lhs = fsb[:, ip * 128:(ip + 1) * 128]
                        nc.tensor.matmul(psA[:, ipl * 192:(ipl + 1) * 192],
                                         lhs, cs["ExF"][:],
                                         start=True, stop=True)
                    for k in range(4):
                        ch = 4 * ip2 + k
                        nc.tensor.matmul(psT1[:, k * 128:(k + 1) * 128],
                                         fsb[:, ch * 64:(ch + 1) * 64],
                                         cs["RxT"][:],
                                         start=True, stop=True)
                    a_t = wk.tile([128, 384], BF16, tag=f"at{ip2}",
                                  name=f"at{ip2}")
                    drain('d_at', a_t[:], psA[:], b)
                    a_ts.append(a_t)
                    t1 = wk.tile([64, 512], BF16, tag=f"t1{ip2}",
                                 name=f"t1{ip2}")
                    drain('d_t1', t1[:], psT1[:], b)
                    t1s.append(t1)
                s['a_ts'] = a_ts
                s['t1s'] = t1s

            def emit_stage2(b):
                s = st[b]
                psF = [pp.tile([128, 128], F32, tag=f"bankF{h}",
                               name=f"psF{h}") for h in range(2)]
                for i in range(8):
                    a_t = s['a_ts'][i // 4]
                    base = ((i // 2) % 2) * 192
                    po = (i % 2) * 64
                    A_RI = a_t[po:po + 64, base:base + 128]
                    A_IS = a_t[po:po + 64, base + 64:base + 192]
                    h, im = i // 4, i % 4
                    sl = slice(im * 32, (im + 1) * 32)
                    tp = (po, im * 32)
                    nc.tensor.matmul(psF[h][sl, :],
                                     cs["EyCT"][po:po + 64, :], A_RI,
                                     start=True, stop=False, tile_position=tp)
                    nc.tensor.matmul(psF[h][sl, :],
                                     cs["EyST"][po:po + 64, :], A_IS,
                                     start=False, stop=True, tile_position=tp)
                Fcv = wk.tile([128, 256], BF16, tag="Fcv", name="Fcv")
                for h in range(2):
                    drain('d_fcv', Fcv[:, h * 128:(h + 1) * 128], psF[h][:])
                s['Fcv'] = Fcv

            def emit_conv(b):
                s = st[b]
                Fcv = s['Fcv']
                Mw = []
                for RI in range(2):
                    m_t = wp.tile([128, 2048], BF16, tag=f"mw{RI}", bufs=2,
                                  name=f"mw{RI}")
                    in0 = _view(Fcv[:], RI * 64,
                                [Fcv[:].ap[0], [0, 16], [128, 2], [1, 64]])
                    tt_eng(2048, cfg['mw_eng']).tensor_mul(
                        m_t[:].rearrange("p (j h f) -> p j h f", j=16, h=2),
                        in0,
                        k_sb[:].rearrange("p (j h f) -> p j h f", j=16, h=2))
                    Mw.append(m_t)

                Bu = wk.tile([64, 1024], BF16, tag="Bu", name="Bu")
                Bv = wk.tile([64, 1024], BF16, tag="Bv", name="Bv")
                BuR, BuI = Bu[0:32, :], Bu[32:64, :]
                BvR, BvI = Bv[0:32, :], Bv[32:64, :]
                tsg = cs["tsg"]
                for RI in range(2):
                    for jh in range(2):
                        ps_acv = pp.tile([32, 512], F32, tag="bankA", bufs=2,
                                         name="ps_acv")
                        for h in range(2):
                            rhs = _view(Mw[RI][:], jh * 1024 + h * 64,
                                        [Mw[RI][:].ap[0], [128, 8], [1, 64]])
                            nc.tensor.matmul(ps_acv[:], cs["S_sel"][:], rhs,
                                             start=(h == 0), stop=(h == 1))
                        if cfg.get('acv_direct'):
                            tt_state[0] += 2 * (512 * 1.0417 + 60)
                            beng0 = beng1 = nc.vector
                            src_ap = ps_acv[:]
                        else:
                            acv = wk.tile([32, 512], BF16, tag="acv",
                                          name="acv")
                            drain('d_acv', acv[:], ps_acv[:], b)
                            src_ap = acv[:]
                            beng0 = tt_eng(512, cfg['bubv_eng'])
                            beng1 = tt_eng(512, cfg['bubv_eng'])
                        osl = slice(jh * 512, (jh + 1) * 512)
                        if RI == 0:
                            beng0.tensor_mul(BuI[:, osl], src_ap,
                                             tsg[:, 512:1024])
                            beng1.tensor_mul(BvI[:, osl], src_ap,
                                             tsg[:, 1536:2048])
                        else:
                            beng0.tensor_mul(BuR[:, osl], src_ap,
                                             tsg[:, 0:512])
                            beng1.tensor_mul(BvR[:, osl], src_ap,
                                             tsg[:, 1024:1536])
                s['B'] = (Bu, Bv)

            def emit_staging(b):
                s = st[b]
                sts = []
                for ci in range(len(CH_SPLITS) - 1):
                    ncols = (CH_SPLITS[ci + 1] - CH_SPLITS[ci]) * 128
                    stt = stp.tile([128, ncols], BF16, tag=f"st{ci}",
                                   name=f"st{ci}")
                    if cfg.get('level', 99) < 4:
                        nc.vector.memset(stt[:], 0.0)
                    sts.append(stt)
                s['sts'] = sts

            def emit_fr(b):
                s = st[b]
                sts = s['sts']
                for ip2 in range(2):
                    psUf = pp.tile([128, 512], F32, tag=f"bankF{2 + ip2}",
                                   name="psUf")
                    for k in range(4):
                        nc.tensor.matmul(
                            psUf[:, k * 128:(k + 1) * 128],
                            s['t1s'][ip2][0:64, k * 128:(k + 1) * 128],
                            cs["CyT"][0:64, :],
                            start=True, stop=True)
                    drain('d_fr', sts[0][:, ip2 * 512:(ip2 + 1) * 512],
                          psUf[:], b)

            ps_tick = [0]

            def emit_synth(b):
                s = st[b]
                Bu, Bv = s['B']
                u_q, v_q = [], []
                for q in range(4):
                    for B_, dst_list in ((Bu, u_q), (Bv, v_q)):
                        ps_tick[0] += 1
                        psG = pp.tile([128, 512], F32,
                                      tag=f"bankF{2 + ps_tick[0] % 2}",
                                      name="psG")
                        for chl in range(4):
                            ch = q * 4 + chl
                            lhsT = B_[0:64, ch * 64:(ch + 1) * 64]
                            ccol = slice(chl * 128, (chl + 1) * 128)
                            nc.tensor.matmul(
                                psG[0:64, ccol], lhsT, cs["QFRs"],
                                start=True, stop=True,
                                tile_position=(0, 0))
                            nc.tensor.matmul(
                                psG[64:128, ccol], lhsT, cs["QFIs"],
                                start=True, stop=True,
                                tile_position=(0, 64))
                        gsb = wk.tile([128, 512], BF16, tag="gsb",
                                      name="gsb", bufs=3)
                        drain('d_g', gsb[:], psG[:], b)
                        psU = pp.tile([128, 512], F32,
                                      tag=f"bankF{4 + ps_tick[0] % 2}",
                                      name="psU")
                        for chloc in range(4):
                            nc.tensor.matmul(
                                psU[:, chloc * 128:(chloc + 1) * 128],
                                cs["PRTPnIT"],
                                gsb[:, chloc * 128:(chloc + 1) * 128],
                                start=True, stop=True)
                        nm = ('u' if dst_list is u_q else 'v') + f"q{q}"
                        qt = uvp.tile([128, 512], BF16, tag=nm, name=nm)
                        drain('d_psu', qt[:], psU[:], b)
                        dst_list.append(qt)
                s['u_q'] = u_q
                s['v_q'] = v_q

            def emit_cross(b):
                s = st[b]
                u_q, v_q, sts = s['u_q'], s['v_q'], s['sts']

                def st_sub(a, b0, cnt, in0, in1):
                    pch = PAIR_BASE[a] + (b0 - a - 1)
                    ci = max(i for i in range(len(CH_SPLITS) - 1)
                             if CH_SPLITS[i] <= pch)
                    assert pch + cnt <= CH_SPLITS[ci + 1], (a, b0, cnt)
                    out = sts[ci][:, (pch - CH_SPLITS[ci]) * 128:
                                  (pch - CH_SPLITS[ci] + cnt) * 128]
                    tt_eng(cnt * 128).tensor_sub(
                        out.rearrange("p (c y) -> p c y", c=cnt), in0, in1)

                for gI in range(4):
                    for gJ in range(gI, 4):
                        W1 = wp.tile([128, 2048], BF16, tag="W1", bufs=3,
                                     name="W1")
                        for ai in range(4):
                            out = W1[:, ai * 512:(ai + 1) * 512].rearrange(
                                "p (c y) -> p c y", c=4)
                            in0 = _bcast(
                                u_q[gI][:, ai * 128:(ai + 1) * 128], 4)
                            in1 = v_q[gJ][:].rearrange(
                                "p (c y) -> p c y", c=4)
                            tt_eng(512).tensor_mul(out, in0, in1)
                        if gI != gJ:
                            W2 = wp.tile([128, 2048], BF16, tag="W2", bufs=3,
                                         name="W2")
                            for bj in range(4):
                                out = W2[:, bj * 512:(bj + 1) * 512].rearrange(
                                    "p (c y) -> p c y", c=4)
                                in0 = _bcast(
                                    u_q[gJ][:, bj * 128:(bj + 1) * 128], 4)
                                in1 = v_q[gI][:].rearrange(
                                    "p (c y) -> p c y", c=4)
                                tt_eng(512).tensor_mul(out, in0, in1)
                            for ai in range(4):
                                a = 4 * gI + ai
                                in0 = _view(W1[:], ai * 512,
                                            [W1[:].ap[0], [128, 4], [1, 128]])
                                in1 = _view(W2[:], ai * 128,
                                            [W2[:].ap[0], [512, 4], [1, 128]])
                                st_sub(a, 4 * gJ, 4, in0, in1)
                        else:
                            for ai in range(3):
                                a = 4 * gI + ai
                                cnt = 3 - ai
                                in0 = _view(W1[:], ai * 512 + (ai + 1) * 128,
                                            [W1[:].ap[0], [128, cnt],
                                             [1, 128]])
                                in1 = _view(W1[:], (ai + 1) * 512 + ai * 128,
                                            [W1[:].ap[0], [512, cnt],
                                             [1, 128]])
                                st_sub(a, a + 1, cnt, in0, in1)

            def emit_dma(b):
                s = st[b]
                for ci in range(len(CH_SPLITS) - 1):
                    c0, c1 = CH_SPLITS[ci], CH_SPLITS[ci + 1]
                    eng = qmap[cfg['dma_chunks'][b][ci]]
                    eng.dma_start(
                        out=out_sh[b, :, c0:c1, :],
                        in_=s['sts'][ci][:].rearrange("p (c y) -> p c y",
                                                      c=c1 - c0))

            lvl = cfg.get('level', 99)
            phases = [emit_staging, emit_stage1, emit_stage2]
            if lvl >= 1:
                phases.append(emit_conv)
            if lvl >= 2:
                phases.append(emit_fr)
            if lvl >= 3:
                phases.append(emit_synth)
            for rep in range(reps):
                for ph in phases:
                    for b in range(B_PER_CORE):
                        ph(b)
                for b in range(B_PER_CORE):
                    if lvl >= 4:
                        emit_cross(b)
                    emit_dma(b)
    nc.compile()
    return nc


_PROGRAM = {}


def _get_program(reps=1, **kw):
    key = (reps, tuple(sorted(kw.items())))
    if key not in _PROGRAM:
        _PROGRAM[key] = build_program(reps, **kw)
    return _PROGRAM[key]


LAST_EXEC_NS = None
LAST_RESULT = None


def kernel(f, kernel):
    global LAST_EXEC_NS, LAST_RESULT
    import os
    f = np.ascontiguousarray(f, dtype=np.float32)
    k_all = _prep_k_all(np.asarray(kernel))
    nc = _get_program()
    in_maps = [
        {"f_in": f[2 * c:2 * c + 2], "k_all": k_all} for c in range(N_CORES)
    ]
    trace = bool(os.environ.get("KERNEL_TRACE"))
    res = run_bass_kernel_spmd(nc, in_maps, list(range(N_CORES)), trace=trace)
    LAST_RESULT = res
    if res.exec_time_ns is not None:
        LAST_EXEC_NS = res.exec_time_ns
    out = np.concatenate(
        [np.asarray(res.results[c]["out_sh"]).astype(np.float32)
         for c in range(N_CORES)], axis=0)
    return out.transpose(0, 2, 1, 3)


# revision 13
# speedup vs baseline: 2.1985x; 1.0243x over previous
"""Trainium2 Bass kernel for nn_EquivariantLayer — bf16 redesign.

Data-parallel over batch (2 samples/core x 8 cores). All DFTs are matmuls
on the TensorEngine in bf16 (1 cyc/row vs 4 for f32). Cross products on
DVE/Pool in bf16 (2x DVE mode). Output staged per-sample in SBUF (bf16)
and stored with 3 chunk-DMAs per sample over the 3 DMA queues
(SP / Activation / Pool). Host casts bf16 -> f32.

Per sample:
  fsb    = cast-load f (Pool SWDGE, f32->bf16)      [64, (i8,y64)]
  psA    = fsb^T @ ExF    (x-DFT)                   [128, (RI,kx) 192]
  psT1   = fsb^T @ RxT    (fr row transform)        [128, X 128]
  Fcv    = y-DFT (EyC/EyS accumulate)               [128, (h,RI,kx) 256]
  Mw[RI] = Fcv (x) k_sb   (conv products)           [128, 2048]
  ps_acv = S_sel @ Mw     (i-reduction)             [32, 512] x4
  Bu/Bv  = acv (x) tsg    (uncurl multipliers)      [32, 1024] x4
  psG    = B @ QF1 + B_I @ QF2  (ky-inverse)        [128=(ch2,kx), (RI,Y) 256]
  Gsb    = drain psG (1 copy)
  psU    = PRT64 @ G_R - PIT64 @ G_I per ch (kx-inverse, accumulated
           into column blocks, 4 ch per PSUM bank)  [128, 512]
  u_s/v_s fields bf16; fr direct path via CyT
  W      = u_a * v_b all 16x16 ordered products     [128, 32768] bf16
  subs   -> staging tiles st0/st1/st2 (bf16), ch-grouped
  DMA    st* -> out_sh[b] on SP/ACT/POOL queues
"""
import sys
import numpy as np
import ml_dtypes

if '/opt/trn_rl_repo' not in sys.path:
    sys.path.insert(0, '/opt/trn_rl_repo')

import concourse.bass as bass
from concourse import bacc
import concourse.mybir as mybir
import concourse.tile as tile
from concourse.bass import AP
from concourse.bass_utils import run_bass_kernel_spmd

F32 = mybir.dt.float32
BF16 = mybir.dt.bfloat16
N_CORES = 8
B_PER_CORE = 2
C1, C2, N1, N2 = 8, 16, 64, 128
NCH_OUT = 128

PAIR_BASE = {}
_p = 8
for _a in range(15):
    PAIR_BASE[_a] = _p
    _p += 15 - _a
assert _p == 128

CH_SPLITS = [0, 37, 62, 83, 100, 113, 128]  # a-run aligned chunk bounds


def _bf16(a):
    return np.ascontiguousarray(np.asarray(a, dtype=np.float32),
                                dtype=ml_dtypes.bfloat16)


def _host_consts():
    x = np.arange(64)
    kx = np.arange(64)
    c = np.arange(32)
    y = np.arange(64)
    X = np.arange(128)
    Y = np.arange(128)

    FRs = np.where(kx <= 32, kx, kx - 64).astype(np.float64)

    ExR = np.cos(2 * np.pi * np.outer(kx, x) / 64)
    ExI = -np.sin(2 * np.pi * np.outer(kx, x) / 64)
    ExF = np.concatenate([ExR.T, ExI.T, -ExR.T], axis=1)     # [x, 192]

    EyCT = np.cos(2 * np.pi * np.outer(c, y) / 64).T          # [y, 32]
    EyST = np.sin(2 * np.pi * np.outer(c, y) / 64).T

    S_sel = np.zeros((128, 32))
    for im in range(4):
        S_sel[im * 32 + np.arange(32), np.arange(32)] = 1.0

    den = FRs[None, :] ** 2 + c[:, None].astype(np.float64) ** 2
    den[0, 0] = 1.0
    t_u = c[:, None] / den                            # [32, 64]
    s_v = -FRs[None, :] / den
    t_rep = np.tile(t_u, (1, 8))                      # [32, 512]
    s_rep = np.tile(s_v, (1, 8))
    tsg = np.concatenate([-t_rep, t_rep, -s_rep, s_rep], axis=1)  # [32, 2048]

    w_c = np.where(c == 0, 1.0, 2.0)
    s_q = 2.0 / (128.0 * 128.0)
    QRT = (s_q * w_c[None, :] * np.cos(2 * np.pi * np.outer(Y, c) / 128)).T
    QIT = (s_q * w_c[None, :] * np.sin(2 * np.pi * np.outer(Y, c) / 128)).T
    QFRs = np.concatenate([QRT, -QIT], axis=0)        # [64, 128] K-stack
    QFIs = np.concatenate([QIT, QRT], axis=0)

    PRT = np.cos(2 * np.pi * np.outer(FRs, X) / 128)  # [64, 128]
    PIT = np.sin(2 * np.pi * np.outer(FRs, X) / 128)
    PRT[32, :] = 0.0
    PIT[32, :] = 0.0
    PRTPnIT = np.concatenate([PRT, -PIT], axis=0)     # [128, 128] K-stack

    # fr direct path
    EyRm = np.cos(2 * np.pi * np.outer(c, y) / 64)
    EyIm = -np.sin(2 * np.pi * np.outer(c, y) / 64)
    QRm = s_q * w_c[None, :] * np.cos(2 * np.pi * np.outer(Y, c) / 128)
    QIm = s_q * w_c[None, :] * np.sin(2 * np.pi * np.outer(Y, c) / 128)
    Rx = PRT.T @ ExR - PIT.T @ ExI                    # [128, 64]
    Cy = QRm @ EyRm - QIm @ EyIm                      # [128, 64]
    RxT = Rx.T                                        # [64, 128]
    CyT = np.concatenate([Cy.T, Cy.T], axis=0)        # [128, 128]

    dup = lambda m: np.concatenate([m, m], axis=0)   # both partition halves
    return dict(ExF=_bf16(ExF), EyCT=_bf16(dup(EyCT)), EyST=_bf16(dup(EyST)),
                S_sel=_bf16(S_sel), tsg=_bf16(tsg), QF1=_bf16(QF1),
                QF2=_bf16(QF2), PRT64=_bf16(dup(PRT)), nPIT64=_bf16(dup(nPIT)),
                RxT=_bf16(RxT), CyT=_bf16(CyT))


def _rot90_kernel(k):
    yk = np.swapaxes(k, -2, -1)
    return np.concatenate([yk[..., :1], yk[..., :0:-1]], axis=-1)


def _symmetric_kernel(k):
    k1 = k
    k2 = _rot90_kernel(k1)
    k3 = _rot90_kernel(k2)
    k4 = _rot90_kernel(k3)
    k5 = np.swapaxes(k1, -2, -1)
    k6 = _rot90_kernel(k5)
    k7 = _rot90_kernel(k6)
    k8 = _rot90_kernel(k7)
    return (k1 + k2 + k3 + k4 + k5 + k6 + k7 + k8) / 8.0


def _prep_k_all(kernel_np):
    """kernel [1,8,16,64,64] -> k_all [128, 2048] conv-layout, bf16."""
    ksym = _symmetric_kernel(kernel_np.astype(np.float64))[0]
    K = np.fft.rfft2(ksym).real                                 # [8,16,64,33]
    Kc = np.transpose(K[:, :, :, :32], (0, 1, 3, 2)).copy()     # [i,j,c,kx]
    Kc[:, :, :, 32] = 0.0
    k_all = np.zeros((128, 2048), dtype=np.float64)
    for i in range(8):
        h, im = i // 4, i % 4
        for j in range(16):
            k_all[im * 32:(im + 1) * 32,
                  j * 128 + h * 64: j * 128 + h * 64 + 64] = Kc[i, j]
    return _bf16(k_all)


def _bcast(ap, n, axis_pos=1):
    dims = list(ap.ap)
    dims.insert(axis_pos, [0, n])
    return AP(ap.tensor, ap.offset, dims)


def _view(ap, offset_elems, dims):
    return AP(ap.tensor, ap.offset + offset_elems, dims)


DEFAULT_CFG = dict(
    d_at='act', d_t1='act', d_fcv='act', d_acv='act', d_g='act',
    d_psu='act', d_fr='act',
    d_at0='dve', d_t10='dve', d_fcv0='dve',
    acv_direct=True,
    mw_eng=None, bubv_eng=None,
    tt_pool_frac=0.48,       # relative weight of Pool in the TT split
    dma_chunks=(('sp', 'act') * 3, ('act', 'sp') * 3),
)


def build_program(reps=1, **cfg_over):
    cfg = dict(DEFAULT_CFG)
    cfg.update(cfg_over)
    nc = bacc.Bacc("TRN2", target_bir_lowering=False)
    consts = _host_consts()

    f_in = nc.dram_tensor("f_in", [B_PER_CORE, C1, 64, 64], F32,
                          kind="ExternalInput")
    k_in = nc.dram_tensor("k_all", [128, 2048], BF16, kind="ExternalInput")
    out_sh = nc.dram_tensor("out_sh", [B_PER_CORE, 128, NCH_OUT, 128], BF16,
                            kind="ExternalOutput")

    cdr = {n: nc.inline_tensor(a, name=f"c_{n}") for n, a in consts.items()}

    mix_tick = [0]

    def drain(which, out_ap, in_ap):
        e = cfg[which]
        if e == 'mix':
            mix_tick[0] += 1
            e = 'dve' if mix_tick[0] % 2 else 'act'
        if e == 'dve':
            nc.vector.tensor_copy(out_ap, in_ap)
        else:
            nc.scalar.copy(out=out_ap, in_=in_ap)

    # weighted greedy balance of TT ops between DVE and Pool
    tt_state = [0.0, 0.0]       # projected ns on dve, pool

    def tt_eng(fe, pref=None):
        if pref == 'dve':
            tt_state[0] += fe * 0.521 + 60
            return nc.vector
        if pref == 'gps':
            tt_state[1] += fe * 0.833 + 25
            return nc.gpsimd
        w_pool = cfg['tt_pool_frac']
        t_d = (tt_state[0] + fe * 0.521 + 60) / max(1.0 - w_pool, 1e-6)
        t_p = (tt_state[1] + fe * 0.833 + 25) / max(w_pool, 1e-6)
        if t_p < t_d:
            tt_state[1] += fe * 0.833 + 25
            return nc.gpsimd
        tt_state[0] += fe * 0.521 + 60
        return nc.vector

    with tile.TileContext(nc) as tc:
        with (
            tc.tile_pool(name="cp", bufs=1) as cp,
            tc.tile_pool(name="wk", bufs=2) as wk,
            tc.tile_pool(name="uv", bufs=2) as uvp,
            tc.tile_pool(name="wp", bufs=1) as wp,
            tc.tile_pool(name="stp", bufs=2) as stp,
            tc.tile_pool(name="pp", bufs=1, space="PSUM") as pp,
        ):
            # ---- loads: f first (needed earliest), consts spread ----
            st = {b: {} for b in range(B_PER_CORE)}
            for b in range(B_PER_CORE):
                fsb32 = wk.tile([64, 512], F32, tag="fsb32", name="fsb32")
                nc.sync.dma_start(
                    out=fsb32[:].rearrange("x (i y) -> x i y", i=8),
                    in_=f_in[b].rearrange("i x y -> x i y"))
                fsb = wk.tile([64, 512], BF16, tag="fsb", name="fsb")
                nc.vector.tensor_copy(fsb[:], fsb32[:])
                st[b]['fsb'] = fsb

            cs = {}
            lq = [nc.sync, nc.scalar]
            order = ["ExF", "RxT", "EyCT", "EyST", "S_sel", "CyT",
                     "tsg", "QF1", "QF2", "PRT64", "nPIT64"]
            for li, name in enumerate(order):
                arr = consts[name]
                t = cp.tile(list(arr.shape), BF16, tag=f"c_{name}",
                            name=f"cs_{name}")
                lq[li % 2].dma_start(out=t[:], in_=cdr[name][:])
                cs[name] = t
            k_sb = cp.tile([128, 2048], BF16, tag="k_sb", name="k_sb")
            nc.scalar.dma_start(out=k_sb[:], in_=k_in[:])

            qmap = {'sp': nc.sync, 'act': nc.scalar, 'gps': nc.gpsimd}

            def emit_stage1(b):
                s = st[b]
                fsb = s['fsb']
                a_ts, t1s = [], []
                for ip2 in range(2):
                    psA = pp.tile([128, 384], F32, tag="bankA", bufs=2,
                                  name="psA")
                    psT1 = pp.tile([64, 512], F32, tag="bankA", bufs=2,
                                   name="psT1")
                    for ipl in range(2):
                        ip = 2 * ip2 + ipl
                        lhs = fsb[:, ip * 128:(ip + 1) * 128]
                        nc.tensor.matmul(psA[:, ipl * 192:(ipl + 1) * 192],
                                         lhs, cs["ExF"][:],
                                         start=True, stop=True)
                    for k in range(4):
                        ch = 4 * ip2 + k
                        nc.tensor.matmul(psT1[:, k * 128:(k + 1) * 128],
                                         fsb[:, ch * 64:(ch + 1) * 64],
                                         cs["RxT"][:],
                                         start=True, stop=True)
                    a_t = wk.tile([128, 384], BF16, tag=f"at{ip2}",
                                  name=f"at{ip2}")
                    drain('d_at', a_t[:], psA[:], b)
                    a_ts.append(a_t)
                    t1 = wk.tile([64, 512], BF16, tag=f"t1{ip2}",
                                 name=f"t1{ip2}")
                    drain('d_t1', t1[:], psT1[:], b)
                    t1s.append(t1)
                s['a_ts'] = a_ts
                s['t1s'] = t1s

            def emit_stage2(b):
                s = st[b]
                psF = [pp.tile([128, 128], F32, tag=f"bankF{h}",
                               name=f"psF{h}") for h in range(2)]
                for i in range(8):
                    a_t = s['a_ts'][i // 4]
                    base = ((i // 2) % 2) * 192
                    po = (i % 2) * 64
                    A_RI = a_t[po:po + 64, base:base + 128]
                    A_IS = a_t[po:po + 64, base + 64:base + 192]
                    h, im = i // 4, i % 4
                    sl = slice(im * 32, (im + 1) * 32)
                    tp = (po, im * 32)
                    nc.tensor.matmul(psF[h][sl, :],
                                     cs["EyCT"][po:po + 64, :], A_RI,
                                     start=True, stop=False, tile_position=tp)
                    nc.tensor.matmul(psF[h][sl, :],
                                     cs["EyST"][po:po + 64, :], A_IS,
                                     start=False, stop=True, tile_position=tp)
                Fcv = wk.tile([128, 256], BF16, tag="Fcv", name="Fcv")
                for h in range(2):
                    drain('d_fcv', Fcv[:, h * 128:(h + 1) * 128], psF[h][:])
                s['Fcv'] = Fcv

            def emit_conv(b):
                s = st[b]
                Fcv = s['Fcv']
                Mw = []
                for RI in range(2):
                    m_t = wp.tile([128, 2048], BF16, tag=f"mw{RI}", bufs=2,
                                  name=f"mw{RI}")
                    in0 = _view(Fcv[:], RI * 64,
                                [Fcv[:].ap[0], [0, 16], [128, 2], [1, 64]])
                    tt_eng(2048, cfg['mw_eng']).tensor_mul(
                        m_t[:].rearrange("p (j h f) -> p j h f", j=16, h=2),
                        in0,
                        k_sb[:].rearrange("p (j h f) -> p j h f", j=16, h=2))
                    Mw.append(m_t)

                Bu = wk.tile([64, 1024], BF16, tag="Bu", name="Bu")
                Bv = wk.tile([64, 1024], BF16, tag="Bv", name="Bv")
                BuR, BuI = Bu[0:32, :], Bu[32:64, :]
                BvR, BvI = Bv[0:32, :], Bv[32:64, :]
                tsg = cs["tsg"]
                for RI in range(2):
                    for jh in range(2):
                        ps_acv = pp.tile([32, 512], F32, tag="bankA", bufs=2,
                                         name="ps_acv")
                        for h in range(2):
                            rhs = _view(Mw[RI][:], jh * 1024 + h * 64,
                                        [Mw[RI][:].ap[0], [128, 8], [1, 64]])
                            nc.tensor.matmul(ps_acv[:], cs["S_sel"][:], rhs,
                                             start=(h == 0), stop=(h == 1))
                        if cfg.get('acv_direct'):
                            tt_state[0] += 2 * (512 * 1.0417 + 60)
                            beng0 = beng1 = nc.vector
                            src_ap = ps_acv[:]
                        else:
                            acv = wk.tile([32, 512], BF16, tag="acv",
                                          name="acv")
                            drain('d_acv', acv[:], ps_acv[:], b)
                            src_ap = acv[:]
                            beng0 = tt_eng(512, cfg['bubv_eng'])
                            beng1 = tt_eng(512, cfg['bubv_eng'])
                        osl = slice(jh * 512, (jh + 1) * 512)
                        if RI == 0:
                            beng0.tensor_mul(BuI[:, osl], src_ap,
                                             tsg[:, 512:1024])
                            beng1.tensor_mul(BvI[:, osl], src_ap,
                                             tsg[:, 1536:2048])
                        else:
                            beng0.tensor_mul(BuR[:, osl], src_ap,
                                             tsg[:, 0:512])
                            beng1.tensor_mul(BvR[:, osl], src_ap,
                                             tsg[:, 1024:1536])
                s['B'] = (Bu, Bv)

            def emit_staging(b):
                s = st[b]
                sts = []
                for ci in range(len(CH_SPLITS) - 1):
                    ncols = (CH_SPLITS[ci + 1] - CH_SPLITS[ci]) * 128
                    stt = stp.tile([128, ncols], BF16, tag=f"st{ci}",
                                   name=f"st{ci}")
                    if cfg.get('level', 99) < 4:
                        nc.vector.memset(stt[:], 0.0)
                    sts.append(stt)
                s['sts'] = sts

            def emit_fr(b):
                s = st[b]
                sts = s['sts']
                for ip2 in range(2):
                    psUf = pp.tile([128, 512], F32, tag=f"bankF{2 + ip2}",
                                   name="psUf")
                    for k in range(4):
                        nc.tensor.matmul(
                            psUf[:, k * 128:(k + 1) * 128],
                            s['t1s'][ip2][0:64, k * 128:(k + 1) * 128],
                            cs["CyT"][0:64, :],
                            start=True, stop=True)
                    drain('d_fr', sts[0][:, ip2 * 512:(ip2 + 1) * 512],
                          psUf[:], b)

            ps_tick = [0]

            def emit_synth(b):
                s = st[b]
                Bu, Bv = s['B']
                u_q, v_q = [], []
                for q in range(4):
                    for B_, dst_list in ((Bu, u_q), (Bv, v_q)):
                        ps_tick[0] += 1
                        psG = pp.tile([128, 512], F32,
                                      tag=f"bankF{2 + ps_tick[0] % 2}",
                                      name="psG")
                        for chl in range(4):
                            ch = q * 4 + chl
                            lhsT = B_[0:64, ch * 64:(ch + 1) * 64]
                            ccol = slice(chl * 128, (chl + 1) * 128)
                            nc.tensor.matmul(
                                psG[0:64, ccol], lhsT, cs["QFRs"],
                                start=True, stop=True,
                                tile_position=(0, 0))
                            nc.tensor.matmul(
                                psG[64:128, ccol], lhsT, cs["QFIs"],
                                start=True, stop=True,
                                tile_position=(0, 64))
                        gsb = wk.tile([128, 512], BF16, tag="gsb",
                                      name="gsb", bufs=3)
                        drain('d_g', gsb[:], psG[:], b)
                        psU = pp.tile([128, 512], F32,
                                      tag=f"bankF{4 + ps_tick[0] % 2}",
                                      name="psU")
                        for chloc in range(4):
                            nc.tensor.matmul(
                                psU[:, chloc * 128:(chloc + 1) * 128],
                                cs["PRTPnIT"],
                                gsb[:, chloc * 128:(chloc + 1) * 128],
                                start=True, stop=True)
                        nm = ('u' if dst_list is u_q else 'v') + f"q{q}"
                        qt = uvp.tile([128, 512], BF16, tag=nm, name=nm)
                        drain('d_psu', qt[:], psU[:], b)
                        dst_list.append(qt)
                s['u_q'] = u_q
                s['v_q'] = v_q

            def emit_cross(b):
                s = st[b]
                u_q, v_q, sts = s['u_q'], s['v_q'], s['sts']

                def st_sub(a, b0, cnt, in0, in1):
                    pch = PAIR_BASE[a] + (b0 - a - 1)
                    ci = max(i for i in range(len(CH_SPLITS) - 1)
                             if CH_SPLITS[i] <= pch)
                    assert pch + cnt <= CH_SPLITS[ci + 1], (a, b0, cnt)
                    out = sts[ci][:, (pch - CH_SPLITS[ci]) * 128:
                                  (pch - CH_SPLITS[ci] + cnt) * 128]
                    tt_eng(cnt * 128).tensor_sub(
                        out.rearrange("p (c y) -> p c y", c=cnt), in0, in1)

                def prod_block(W, uq, vq):
                    # W[p, a, b, y] = u_a * v_b in ONE instruction
                    out = W[:].rearrange("p (a b y) -> p a b y", a=4, b=4)
                    in0 = _view(uq[:], 0,
                                [uq[:].ap[0], [128, 4], [0, 4], [1, 128]])
                    in1 = _view(vq[:], 0,
                                [vq[:].ap[0], [0, 4], [128, 4], [1, 128]])
                    tt_eng(2048).tensor_mul(out, in0, in1)

                for gI in range(4):
                    for gJ in range(gI, 4):
                        W1 = wp.tile([128, 2048], BF16, tag="W1", bufs=3,
                                     name="W1")
                        prod_block(W1, u_q[gI], v_q[gJ])
                        if gI != gJ:
                            W2 = wp.tile([128, 2048], BF16, tag="W2", bufs=3,
                                         name="W2")
                            prod_block(W2, u_q[gJ], v_q[gI])
                            for ai in range(4):
                                a = 4 * gI + ai
                                in0 = _view(W1[:], ai * 512,
                                            [W1[:].ap[0], [128, 4], [1, 128]])
                                in1 = _view(W2[:], ai * 128,
                                            [W2[:].ap[0], [512, 4], [1, 128]])
                                st_sub(a, 4 * gJ, 4, in0, in1)
                        else:
                            for ai in range(3):
                                a = 4 * gI + ai
                                cnt = 3 - ai
                                in0 = _view(W1[:], ai * 512 + (ai + 1) * 128,
                                            [W1[:].ap[0], [128, cnt],
                                             [1, 128]])
                                in1 = _view(W1[:], (ai + 1) * 512 + ai * 128,
                                            [W1[:].ap[0], [512, cnt],
                                             [1, 128]])
                                st_sub(a, a + 1, cnt, in0, in1)

            def emit_dma(b):
                s = st[b]
                for ci in range(len(CH_SPLITS) - 1):
                    c0, c1 = CH_SPLITS[ci], CH_SPLITS[ci + 1]
                    eng = qmap[cfg['dma_chunks'][b][ci]]
                    eng.dma_start(
                        out=out_sh[b, :, c0:c1, :],
                        in_=s['sts'][ci][:].rearrange("p (c y) -> p c y",
                                                      c=c1 - c0))

            lvl = cfg.get('level', 99)
            phases = [emit_staging, emit_stage1, emit_stage2]
            if lvl >= 1:
                phases.append(emit_conv)
            if lvl >= 2:
                phases.append(emit_fr)
            if lvl >= 3:
                phases.append(emit_synth)
            for rep in range(reps):
                for ph in phases:
                    for b in range(B_PER_CORE):
                        ph(b)
                for b in range(B_PER_CORE):
                    if lvl >= 4:
                        emit_cross(b)
                    emit_dma(b)
    nc.compile()
    return nc


_PROGRAM = {}


def _get_program(reps=1, **kw):
    key = (reps, tuple(sorted(kw.items())))
    if key not in _PROGRAM:
        _PROGRAM[key] = build_program(reps, **kw)
    return _PROGRAM[key]


LAST_EXEC_NS = None
LAST_RESULT = None


def kernel(f, kernel):
    global LAST_EXEC_NS, LAST_RESULT
    import os
    f = np.ascontiguousarray(f, dtype=np.float32)
    k_all = _prep_k_all(np.asarray(kernel))
    nc = _get_program()
    in_maps = [
        {"f_in": f[2 * c:2 * c + 2], "k_all": k_all} for c in range(N_CORES)
    ]
    trace = bool(os.environ.get("KERNEL_TRACE"))
    res = run_bass_kernel_spmd(nc, in_maps, list(range(N_CORES)), trace=trace)
    LAST_RESULT = res
    if res.exec_time_ns is not None:
        LAST_EXEC_NS = res.exec_time_ns
    out = np.concatenate(
        [np.asarray(res.results[c]["out_sh"]).astype(np.float32)
         for c in range(N_CORES)], axis=0)
    return out.transpose(0, 2, 1, 3)


# revision 17
# speedup vs baseline: 2.2034x; 1.0022x over previous
"""Trainium2 Bass kernel for nn_EquivariantLayer — bf16 redesign.

Data-parallel over batch (2 samples/core x 8 cores). All DFTs are matmuls
on the TensorEngine in bf16 (1 cyc/row vs 4 for f32). Cross products on
DVE/Pool in bf16 (2x DVE mode). Output staged per-sample in SBUF (bf16)
and stored with 3 chunk-DMAs per sample over the 3 DMA queues
(SP / Activation / Pool). Host casts bf16 -> f32.

Per sample:
  fsb    = cast-load f (Pool SWDGE, f32->bf16)      [64, (i8,y64)]
  psA    = fsb^T @ ExF    (x-DFT)                   [128, (RI,kx) 192]
  psT1   = fsb^T @ RxT    (fr row transform)        [128, X 128]
  Fcv    = y-DFT (EyC/EyS accumulate)               [128, (h,RI,kx) 256]
  Mw[RI] = Fcv (x) k_sb   (conv products)           [128, 2048]
  ps_acv = S_sel @ Mw     (i-reduction)             [32, 512] x4
  Bu/Bv  = acv (x) tsg    (uncurl multipliers)      [32, 1024] x4
  psG    = B @ QF1 + B_I @ QF2  (ky-inverse)        [128=(ch2,kx), (RI,Y) 256]
  Gsb    = drain psG (1 copy)
  psU    = PRT64 @ G_R - PIT64 @ G_I per ch (kx-inverse, accumulated
           into column blocks, 4 ch per PSUM bank)  [128, 512]
  u_s/v_s fields bf16; fr direct path via CyT
  W      = u_a * v_b all 16x16 ordered products     [128, 32768] bf16
  subs   -> staging tiles st0/st1/st2 (bf16), ch-grouped
  DMA    st* -> out_sh[b] on SP/ACT/POOL queues
"""
import sys
import numpy as np
import ml_dtypes

if '/opt/trn_rl_repo' not in sys.path:
    sys.path.insert(0, '/opt/trn_rl_repo')

import concourse.bass as bass
from concourse import bacc
import concourse.mybir as mybir
import concourse.tile as tile
from concourse.bass import AP
from concourse.bass_utils import run_bass_kernel_spmd

F32 = mybir.dt.float32
BF16 = mybir.dt.bfloat16
N_CORES = 8
B_PER_CORE = 2
C1, C2, N1, N2 = 8, 16, 64, 128
NCH_OUT = 128

PAIR_BASE = {}
_p = 8
for _a in range(15):
    PAIR_BASE[_a] = _p
    _p += 15 - _a
assert _p == 128

CH_SPLITS = [0, 37, 62, 83, 100, 113, 128]  # a-run aligned chunk bounds


def _bf16(a):
    return np.ascontiguousarray(np.asarray(a, dtype=np.float32),
                                dtype=ml_dtypes.bfloat16)


def _host_consts():
    x = np.arange(64)
    kx = np.arange(64)
    c = np.arange(32)
    y = np.arange(64)
    X = np.arange(128)
    Y = np.arange(128)

    FRs = np.where(kx <= 32, kx, kx - 64).astype(np.float64)

    ExR = np.cos(2 * np.pi * np.outer(kx, x) / 64)
    ExI = -np.sin(2 * np.pi * np.outer(kx, x) / 64)
    ExF = np.concatenate([ExR.T, ExI.T, -ExR.T], axis=1)     # [x, 192]

    EyCT = np.cos(2 * np.pi * np.outer(c, y) / 64).T          # [y, 32]
    EyST = np.sin(2 * np.pi * np.outer(c, y) / 64).T

    S_sel = np.zeros((128, 32))
    for im in range(4):
        S_sel[im * 32 + np.arange(32), np.arange(32)] = 1.0

    den = FRs[None, :] ** 2 + c[:, None].astype(np.float64) ** 2
    den[0, 0] = 1.0
    t_u = c[:, None] / den                            # [32, 64]
    s_v = -FRs[None, :] / den
    t_rep = np.tile(t_u, (1, 8))                      # [32, 512]
    s_rep = np.tile(s_v, (1, 8))
    tsg = np.concatenate([-t_rep, t_rep, -s_rep, s_rep], axis=1)  # [32, 2048]

    w_c = np.where(c == 0, 1.0, 2.0)
    s_q = 2.0 / (128.0 * 128.0)
    QRT = (s_q * w_c[None, :] * np.cos(2 * np.pi * np.outer(Y, c) / 128)).T
    QIT = (s_q * w_c[None, :] * np.sin(2 * np.pi * np.outer(Y, c) / 128)).T
    QFRs = np.concatenate([QRT, -QIT], axis=0)        # [64, 128] K-stack
    QFIs = np.concatenate([QIT, QRT], axis=0)

    PRT = np.cos(2 * np.pi * np.outer(FRs, X) / 128)  # [64, 128]
    PIT = np.sin(2 * np.pi * np.outer(FRs, X) / 128)
    PRT[32, :] = 0.0
    PIT[32, :] = 0.0
    PRTPnIT = np.concatenate([PRT, -PIT], axis=0)     # [128, 128] K-stack

    # fr direct path
    EyRm = np.cos(2 * np.pi * np.outer(c, y) / 64)
    EyIm = -np.sin(2 * np.pi * np.outer(c, y) / 64)
    QRm = s_q * w_c[None, :] * np.cos(2 * np.pi * np.outer(Y, c) / 128)
    QIm = s_q * w_c[None, :] * np.sin(2 * np.pi * np.outer(Y, c) / 128)
    Rx = PRT.T @ ExR - PIT.T @ ExI                    # [128, 64]
    Cy = QRm @ EyRm - QIm @ EyIm                      # [128, 64]
    RxT = Rx.T                                        # [64, 128]
    CyT = np.concatenate([Cy.T, Cy.T], axis=0)        # [128, 128]

    dup = lambda m: np.concatenate([m, m], axis=0)   # both partition halves
    return dict(ExF=_bf16(ExF), EyCT=_bf16(dup(EyCT)), EyST=_bf16(dup(EyST)),
                S_sel=_bf16(S_sel), tsg=_bf16(tsg), QF1=_bf16(QF1),
                QF2=_bf16(QF2), PRT64=_bf16(dup(PRT)), nPIT64=_bf16(dup(nPIT)),
                RxT=_bf16(RxT), CyT=_bf16(CyT))


def _rot90_kernel(k):
    yk = np.swapaxes(k, -2, -1)
    return np.concatenate([yk[..., :1], yk[..., :0:-1]], axis=-1)


def _symmetric_kernel(k):
    k1 = k
    k2 = _rot90_kernel(k1)
    k3 = _rot90_kernel(k2)
    k4 = _rot90_kernel(k3)
    k5 = np.swapaxes(k1, -2, -1)
    k6 = _rot90_kernel(k5)
    k7 = _rot90_kernel(k6)
    k8 = _rot90_kernel(k7)
    return (k1 + k2 + k3 + k4 + k5 + k6 + k7 + k8) / 8.0


def _prep_k_all(kernel_np):
    """kernel [1,8,16,64,64] -> k_all [128, 2048] conv-layout, bf16."""
    ksym = _symmetric_kernel(kernel_np.astype(np.float64))[0]
    K = np.fft.rfft2(ksym).real                                 # [8,16,64,33]
    Kc = np.transpose(K[:, :, :, :32], (0, 1, 3, 2)).copy()     # [i,j,c,kx]
    Kc[:, :, :, 32] = 0.0
    k_all = np.zeros((128, 2048), dtype=np.float64)
    for i in range(8):
        h, im = i // 4, i % 4
        for j in range(16):
            k_all[im * 32:(im + 1) * 32,
                  j * 128 + h * 64: j * 128 + h * 64 + 64] = Kc[i, j]
    return _bf16(k_all)


def _bcast(ap, n, axis_pos=1):
    dims = list(ap.ap)
    dims.insert(axis_pos, [0, n])
    return AP(ap.tensor, ap.offset, dims)


def _view(ap, offset_elems, dims):
    return AP(ap.tensor, ap.offset + offset_elems, dims)


DEFAULT_CFG = dict(
    d_at='act', d_t1='act', d_fcv='act', d_acv='act', d_g='act',
    d_psu='act', d_fr='act',
    d_at0='dve', d_t10='dve', d_fcv0='dve',
    acv_direct=True, fr_late=True, sample_major=True,
    mw_eng=None, bubv_eng=None,
    tt_pool_frac=0.48,       # relative weight of Pool in the TT split
    dma_chunks=(('sp', 'act') * 3, ('act', 'sp') * 3),
)


def build_program(reps=1, **cfg_over):
    cfg = dict(DEFAULT_CFG)
    cfg.update(cfg_over)
    nc = bacc.Bacc("TRN2", target_bir_lowering=False)
    consts = _host_consts()

    f_in = nc.dram_tensor("f_in", [B_PER_CORE, C1, 64, 64], F32,
                          kind="ExternalInput")
    k_in = nc.dram_tensor("k_all", [128, 2048], BF16, kind="ExternalInput")
    out_sh = nc.dram_tensor("out_sh", [B_PER_CORE, 128, NCH_OUT, 128], BF16,
                            kind="ExternalOutput")

    cdr = {n: nc.inline_tensor(a, name=f"c_{n}") for n, a in consts.items()}

    mix_tick = [0]

    def drain(which, out_ap, in_ap):
        e = cfg[which]
        if e == 'mix':
            mix_tick[0] += 1
            e = 'dve' if mix_tick[0] % 2 else 'act'
        if e == 'dve':
            nc.vector.tensor_copy(out_ap, in_ap)
        else:
            nc.scalar.copy(out=out_ap, in_=in_ap)

    # weighted greedy balance of TT ops between DVE and Pool
    tt_state = [0.0, 0.0]       # projected ns on dve, pool

    def tt_eng(fe, pref=None):
        if pref == 'dve':
            tt_state[0] += fe * 0.521 + 60
            return nc.vector
        if pref == 'gps':
            tt_state[1] += fe * 0.833 + 25
            return nc.gpsimd
        w_pool = cfg['tt_pool_frac']
        t_d = (tt_state[0] + fe * 0.521 + 60) / max(1.0 - w_pool, 1e-6)
        t_p = (tt_state[1] + fe * 0.833 + 25) / max(w_pool, 1e-6)
        if t_p < t_d:
            tt_state[1] += fe * 0.833 + 25
            return nc.gpsimd
        tt_state[0] += fe * 0.521 + 60
        return nc.vector

    with tile.TileContext(nc) as tc:
        with (
            tc.tile_pool(name="cp", bufs=1) as cp,
            tc.tile_pool(name="wk", bufs=2) as wk,
            tc.tile_pool(name="uv", bufs=2) as uvp,
            tc.tile_pool(name="wp", bufs=1) as wp,
            tc.tile_pool(name="stp", bufs=2) as stp,
            tc.tile_pool(name="pp", bufs=1, space="PSUM") as pp,
        ):
            # ---- loads: f first (needed earliest), consts spread ----
            st = {b: {} for b in range(B_PER_CORE)}
            for b in range(B_PER_CORE):
                fsb32 = wk.tile([64, 512], F32, tag="fsb32", name="fsb32")
                nc.sync.dma_start(
                    out=fsb32[:].rearrange("x (i y) -> x i y", i=8),
                    in_=f_in[b].rearrange("i x y -> x i y"))
                fsb = wk.tile([64, 512], BF16, tag="fsb", name="fsb")
                nc.vector.tensor_copy(fsb[:], fsb32[:])
                st[b]['fsb'] = fsb

            cs = {}
            lq = [nc.sync, nc.scalar]
            order = ["ExF", "RxT", "EyCT", "EyST", "S_sel", "CyT",
                     "tsg", "QF1", "QF2", "PRT64", "nPIT64"]
            for li, name in enumerate(order):
                arr = consts[name]
                t = cp.tile(list(arr.shape), BF16, tag=f"c_{name}",
                            name=f"cs_{name}")
                lq[li % 2].dma_start(out=t[:], in_=cdr[name][:])
                cs[name] = t
            k_sb = cp.tile([128, 2048], BF16, tag="k_sb", name="k_sb")
            nc.scalar.dma_start(out=k_sb[:], in_=k_in[:])

            qmap = {'sp': nc.sync, 'act': nc.scalar, 'gps': nc.gpsimd}

            def emit_stage1(b):
                s = st[b]
                fsb = s['fsb']
                a_ts, t1s = [], []
                for ip2 in range(2):
                    psA = pp.tile([128, 384], F32, tag="bankA", bufs=2,
                                  name="psA")
                    psT1 = pp.tile([64, 512], F32, tag="bankA", bufs=2,
                                   name="psT1")
                    for ipl in range(2):
                        ip = 2 * ip2 + ipl
                        lhs = fsb[:, ip * 128:(ip + 1) * 128]
                        nc.tensor.matmul(psA[:, ipl * 192:(ipl + 1) * 192],
                                         lhs, cs["ExF"][:],
                                         start=True, stop=True)
                    for k in range(4):
                        ch = 4 * ip2 + k
                        nc.tensor.matmul(psT1[:, k * 128:(k + 1) * 128],
                                         fsb[:, ch * 64:(ch + 1) * 64],
                                         cs["RxT"][:],
                                         start=True, stop=True)
                    a_t = wk.tile([128, 384], BF16, tag=f"at{ip2}",
                                  name=f"at{ip2}")
                    drain('d_at', a_t[:], psA[:], b)
                    a_ts.append(a_t)
                    t1 = wk.tile([64, 512], BF16, tag=f"t1{ip2}",
                                 name=f"t1{ip2}")
                    drain('d_t1', t1[:], psT1[:], b)
                    t1s.append(t1)
                s['a_ts'] = a_ts
                s['t1s'] = t1s

            def emit_stage2(b):
                s = st[b]
                psF = [pp.tile([128, 128], F32, tag=f"bankF{h}",
                               name=f"psF{h}") for h in range(2)]
                for i in range(8):
                    a_t = s['a_ts'][i // 4]
                    base = ((i // 2) % 2) * 192
                    po = (i % 2) * 64
                    A_RI = a_t[po:po + 64, base:base + 128]
                    A_IS = a_t[po:po + 64, base + 64:base + 192]
                    h, im = i // 4, i % 4
                    sl = slice(im * 32, (im + 1) * 32)
                    tp = (po, im * 32)
                    nc.tensor.matmul(psF[h][sl, :],
                                     cs["EyCT"][po:po + 64, :], A_RI,
                                     start=True, stop=False, tile_position=tp)
                    nc.tensor.matmul(psF[h][sl, :],
                                     cs["EyST"][po:po + 64, :], A_IS,
                                     start=False, stop=True, tile_position=tp)
                Fcv = wk.tile([128, 256], BF16, tag="Fcv", name="Fcv")
                for h in range(2):
                    drain('d_fcv', Fcv[:, h * 128:(h + 1) * 128], psF[h][:])
                s['Fcv'] = Fcv

            def emit_conv(b):
                s = st[b]
                Fcv = s['Fcv']
                Mw = []
                for RI in range(2):
                    m_t = wp.tile([128, 2048], BF16, tag=f"mw{RI}", bufs=2,
                                  name=f"mw{RI}")
                    in0 = _view(Fcv[:], RI * 64,
                                [Fcv[:].ap[0], [0, 16], [128, 2], [1, 64]])
                    tt_eng(2048, cfg['mw_eng']).tensor_mul(
                        m_t[:].rearrange("p (j h f) -> p j h f", j=16, h=2),
                        in0,
                        k_sb[:].rearrange("p (j h f) -> p j h f", j=16, h=2))
                    Mw.append(m_t)

                Bu = wk.tile([64, 1024], BF16, tag="Bu", name="Bu")
                Bv = wk.tile([64, 1024], BF16, tag="Bv", name="Bv")
                BuR, BuI = Bu[0:32, :], Bu[32:64, :]
                BvR, BvI = Bv[0:32, :], Bv[32:64, :]
                tsg = cs["tsg"]
                for RI in range(2):
                    for jh in range(2):
                        ps_acv = pp.tile([32, 512], F32, tag="bankA", bufs=2,
                                         name="ps_acv")
                        for h in range(2):
                            rhs = _view(Mw[RI][:], jh * 1024 + h * 64,
                                        [Mw[RI][:].ap[0], [128, 8], [1, 64]])
                            nc.tensor.matmul(ps_acv[:], cs["S_sel"][:], rhs,
                                             start=(h == 0), stop=(h == 1))
                        if cfg.get('acv_direct'):
                            tt_state[0] += 2 * (512 * 1.0417 + 60)
                            beng0 = beng1 = nc.vector
                            src_ap = ps_acv[:]
                        else:
                            acv = wk.tile([32, 512], BF16, tag="acv",
                                          name="acv")
                            drain('d_acv', acv[:], ps_acv[:], b)
                            src_ap = acv[:]
                            beng0 = tt_eng(512, cfg['bubv_eng'])
                            beng1 = tt_eng(512, cfg['bubv_eng'])
                        osl = slice(jh * 512, (jh + 1) * 512)
                        if RI == 0:
                            beng0.tensor_mul(BuI[:, osl], src_ap,
                                             tsg[:, 512:1024])
                            beng1.tensor_mul(BvI[:, osl], src_ap,
                                             tsg[:, 1536:2048])
                        else:
                            beng0.tensor_mul(BuR[:, osl], src_ap,
                                             tsg[:, 0:512])
                            beng1.tensor_mul(BvR[:, osl], src_ap,
                                             tsg[:, 1024:1536])
                s['B'] = (Bu, Bv)

            def emit_staging(b):
                s = st[b]
                sts = []
                for ci in range(len(CH_SPLITS) - 1):
                    ncols = (CH_SPLITS[ci + 1] - CH_SPLITS[ci]) * 128
                    stt = stp.tile([128, ncols], BF16, tag=f"st{ci}",
                                   name=f"st{ci}")
                    if cfg.get('level', 99) < 4:
                        nc.vector.memset(stt[:], 0.0)
                    sts.append(stt)
                s['sts'] = sts

            def emit_fr(b):
                s = st[b]
                sts = s['sts']
                for ip2 in range(2):
                    psUf = pp.tile([128, 512], F32, tag=f"bankF{2 + ip2}",
                                   name="psUf")
                    for k in range(4):
                        nc.tensor.matmul(
                            psUf[:, k * 128:(k + 1) * 128],
                            s['t1s'][ip2][0:64, k * 128:(k + 1) * 128],
                            cs["CyT"][0:64, :],
                            start=True, stop=True)
                    drain('d_fr', sts[0][:, ip2 * 512:(ip2 + 1) * 512],
                          psUf[:], b)

            ps_tick = [0]

            def emit_synth(b):
                s = st[b]
                Bu, Bv = s['B']
                u_q, v_q = [], []
                for q in range(4):
                    for B_, dst_list in ((Bu, u_q), (Bv, v_q)):
                        ps_tick[0] += 1
                        psG = pp.tile([128, 512], F32,
                                      tag=f"bankF{2 + ps_tick[0] % 2}",
                                      name="psG")
                        for chl in range(4):
                            ch = q * 4 + chl
                            lhsT = B_[0:64, ch * 64:(ch + 1) * 64]
                            ccol = slice(chl * 128, (chl + 1) * 128)
                            nc.tensor.matmul(
                                psG[0:64, ccol], lhsT, cs["QFRs"],
                                start=True, stop=True,
                                tile_position=(0, 0))
                            nc.tensor.matmul(
                                psG[64:128, ccol], lhsT, cs["QFIs"],
                                start=True, stop=True,
                                tile_position=(0, 64))
                        gsb = wk.tile([128, 512], BF16, tag="gsb",
                                      name="gsb", bufs=3)
                        drain('d_g', gsb[:], psG[:], b)
                        psU = pp.tile([128, 512], F32,
                                      tag=f"bankF{4 + ps_tick[0] % 2}",
                                      name="psU")
                        nc.tensor.matmul(psU[:], cs["PRTPnIT"], gsb[:],
                                         start=True, stop=True)
                        nm = ('u' if dst_list is u_q else 'v') + f"q{q}"
                        qt = uvp.tile([128, 512], BF16, tag=nm, name=nm)
                        drain('d_psu', qt[:], psU[:], b)
                        dst_list.append(qt)
                s['u_q'] = u_q
                s['v_q'] = v_q

            def emit_cross(b):
                s = st[b]
                u_q, v_q, sts = s['u_q'], s['v_q'], s['sts']

                def st_sub(a, b0, cnt, in0, in1):
                    pch = PAIR_BASE[a] + (b0 - a - 1)
                    ci = max(i for i in range(len(CH_SPLITS) - 1)
                             if CH_SPLITS[i] <= pch)
                    assert pch + cnt <= CH_SPLITS[ci + 1], (a, b0, cnt)
                    out = sts[ci][:, (pch - CH_SPLITS[ci]) * 128:
                                  (pch - CH_SPLITS[ci] + cnt) * 128]
                    tt_eng(cnt * 128).tensor_sub(
                        out.rearrange("p (c y) -> p c y", c=cnt), in0, in1)

                def prod_block(W, uq, vq):
                    # W[p, a, b, y] = u_a * v_b in ONE instruction
                    out = W[:].rearrange("p (a b y) -> p a b y", a=4, b=4)
                    in0 = _view(uq[:], 0,
                                [uq[:].ap[0], [128, 4], [0, 4], [1, 128]])
                    in1 = _view(vq[:], 0,
                                [vq[:].ap[0], [0, 4], [128, 4], [1, 128]])
                    tt_eng(2048).tensor_mul(out, in0, in1)

                for gI in range(4):
                    for gJ in range(gI, 4):
                        W1 = wp.tile([128, 2048], BF16, tag="W1", bufs=3,
                                     name="W1")
                        prod_block(W1, u_q[gI], v_q[gJ])
                        if gI != gJ:
                            W2 = wp.tile([128, 2048], BF16, tag="W2", bufs=3,
                                         name="W2")
                            prod_block(W2, u_q[gJ], v_q[gI])
                            for ai in range(4):
                                a = 4 * gI + ai
                                in0 = _view(W1[:], ai * 512,
                                            [W1[:].ap[0], [128, 4], [1, 128]])
                                in1 = _view(W2[:], ai * 128,
                                            [W2[:].ap[0], [512, 4], [1, 128]])
                                st_sub(a, 4 * gJ, 4, in0, in1)
                        else:
                            for ai in range(3):
                                a = 4 * gI + ai
                                cnt = 3 - ai
                                in0 = _view(W1[:], ai * 512 + (ai + 1) * 128,
                                            [W1[:].ap[0], [128, cnt],
                                             [1, 128]])
                                in1 = _view(W1[:], (ai + 1) * 512 + ai * 128,
                                            [W1[:].ap[0], [512, cnt],
                                             [1, 128]])
                                st_sub(a, a + 1, cnt, in0, in1)

            def emit_dma(b):
                s = st[b]
                for ci in range(len(CH_SPLITS) - 1):
                    c0, c1 = CH_SPLITS[ci], CH_SPLITS[ci + 1]
                    eng = qmap[cfg['dma_chunks'][b][ci]]
                    eng.dma_start(
                        out=out_sh[b, :, c0:c1, :],
                        in_=s['sts'][ci][:].rearrange("p (c y) -> p c y",
                                                      c=c1 - c0))

            lvl = cfg.get('level', 99)
            phases = [emit_staging, emit_stage1, emit_stage2]
            mid = []
            if lvl >= 1:
                mid.append(emit_conv)
            if cfg.get('fr_late'):
                if lvl >= 3:
                    mid.append(emit_synth)
                if lvl >= 2:
                    mid.append(emit_fr)
            else:
                if lvl >= 2:
                    mid.append(emit_fr)
                if lvl >= 3:
                    mid.append(emit_synth)
            for rep in range(reps):
                for ph in phases:
                    for b in range(B_PER_CORE):
                        ph(b)
                if cfg.get('sample_major'):
                    for b in range(B_PER_CORE):
                        for ph in mid:
                            ph(b)
                else:
                    for ph in mid:
                        for b in range(B_PER_CORE):
                            ph(b)
                for b in range(B_PER_CORE):
                    if lvl >= 4:
                        emit_cross(b)
                    emit_dma(b)
    nc.compile()
    return nc


_PROGRAM = {}


def _get_program(reps=1, **kw):
    key = (reps, tuple(sorted(kw.items())))
    if key not in _PROGRAM:
        _PROGRAM[key] = build_program(reps, **kw)
    return _PROGRAM[key]


LAST_EXEC_NS = None
LAST_RESULT = None


def kernel(f, kernel):
    global LAST_EXEC_NS, LAST_RESULT
    import os
    f = np.ascontiguousarray(f, dtype=np.float32)
    k_all = _prep_k_all(np.asarray(kernel))
    nc = _get_program()
    in_maps = [
        {"f_in": f[2 * c:2 * c + 2], "k_all": k_all} for c in range(N_CORES)
    ]
    trace = bool(os.environ.get("KERNEL_TRACE"))
    res = run_bass_kernel_spmd(nc, in_maps, list(range(N_CORES)), trace=trace)
    LAST_RESULT = res
    if res.exec_time_ns is not None:
        LAST_EXEC_NS = res.exec_time_ns
    out = np.concatenate(
        [np.asarray(res.results[c]["out_sh"]).astype(np.float32)
         for c in range(N_CORES)], axis=0)
    return out.transpose(0, 2, 1, 3)
